# revision 21
# baseline (speedup 1.0000x reference)
"""Trainium2 Bass kernel for EnhancedTransformerBlock (B=2,T=2048,C=1024,H=16,
SwiGLU HIDDEN=2730, ALiBi-abs + causal attention).

Sharding over 8 cores: batch (2) x head-groups (4 heads/core). Host ships x
both token-major (bf16, LN stats only) and feature-major (bf16, matmul
stream); LN1 is folded into the QKV matmuls via a mean-row augmentation plus
a per-token rstd column scale applied on the PSUM->SBUF copy. Attention runs
with transposed scores S^T[tk,tq]; head pairs share the PE array via row
tiling (K=64 each), the ALiBi key-side bias is folded into V's rows, and the
softmax denominator rides as a scaled ones column through the PV matmul.
Attention output is exchanged with an AllToAll (heads -> token chunks), so
each core projects its own 128-token rows with the full proj matrix - no
reduce needed. The SwiGLU MLP then runs row-parallel with full weights.
Host reassembles row blocks.
"""
import sys, types
sys.path.insert(0, "/opt/trn_rl_repo")
import numpy as np
import ml_dtypes

import concourse.bass as bass
import concourse.tile as tile
from concourse import mybir
import concourse.bass_utils as bass_utils
import bass_rust

# ----------------------------------------------------------------------------
# environment patches (walrus in this container accepts only 1 sync-wait/inst)
# ----------------------------------------------------------------------------
_DRAIN_WAIT_LIMIT = 1

def _patched_drain_and_barrier(self, tick_clock, wait_clock):
    nc = self.nc
    drain_inst = nc.sync.drain()
    wait_clock.add_sem_waits(
        drain_inst.ins, bass_rust.ScopedClock({None: tick_clock.global_clock})
    )
    si = drain_inst.ins.sync_info
    waits = list(si.on_wait) if si is not None else []
    if len(waits) > _DRAIN_WAIT_LIMIT:
        si.on_wait = waits[:_DRAIN_WAIT_LIMIT]
        for i in range(_DRAIN_WAIT_LIMIT, len(waits), _DRAIN_WAIT_LIMIT):
            d2 = nc.sync.drain()
            d2.ins.sync_info = bass_rust.SyncInfo(
                on_wait=waits[i:i + _DRAIN_WAIT_LIMIT], on_update=[]
            )
    nc.all_engine_barrier()
    popped = nc._tile_sem_poison_stack.pop()
    assert popped is self._sem_poison
    nc.clear_and_free_semaphores(list(self.sems.allocated().values()))
    nc.all_engine_barrier()


def _split_excess_waits(nc, limit=_DRAIN_WAIT_LIMIT):
    n = [0]
    for bb in nc.main_func.blocks:
        insts = bb.instructions
        out = []
        changed = False
        for inst in insts:
            si = inst.sync_info
            waits = list(si.on_wait) if si is not None else []
            if len(waits) > limit:
                changed = True
                keep = waits[-limit:]
                rest = waits[:-limit]
                for i in range(0, len(rest), limit):
                    n[0] += 1
                    d = mybir.InstNoOp(
                        name=f"waitsplit-{n[0]}", engine=inst.engine, ins=[], outs=[]
                    )
                    d.sync_info = bass_rust.SyncInfo(
                        on_wait=rest[i:i + limit], on_update=[]
                    )
                    out.append(d)
                si.on_wait = keep
            out.append(inst)
        if changed:
            bb.instructions = out


def _install_patches():
    tile.TileContext._drain_and_barrier = _patched_drain_and_barrier
    if "antenv.axon_hooks" not in sys.modules:
        try:
            from trn_agent_boot.trn_boot import _ntff_profile_via_ctypes
            hook = _ntff_profile_via_ctypes("/opt/axon/libaxon_pjrt.so")
        except Exception:
            hook = None
        mod = types.ModuleType("antenv.axon_hooks")
        mod.get_axon_ntff_profile_hook = lambda: hook
        mod.set_axon_ntff_profile_hook = lambda h: None
        sys.modules["antenv.axon_hooks"] = mod
        bass_utils.upload_artifacts = lambda tmpdir: tmpdir

_install_patches()

# ----------------------------------------------------------------------------
# problem constants
# ----------------------------------------------------------------------------
B, T, C = 2, 2048, 1024
H, D = 16, 64
HID, HIDP = 2730, 2816
N_CORES, TP = 8, 4
ROWS = T // TP          # 512 rows per core after the all-to-all
HL = 4                  # local heads per core
EPS = 1e-5
CBIAS = 24.0
F32, BF16 = mybir.dt.float32, mybir.dt.bfloat16
bf16 = ml_dtypes.bfloat16
ts = bass.ts

NTT = T // 128          # 16 token tiles
NTB = T // 512          # 4 token blocks
CCH = C // 128          # 8 feature chunks
MCH = HIDP // 128       # 22 hidden chunks
RT = ROWS // 128        # 4 row tiles per core
GROUPS = [[0, 1, 2, 3], [4, 5, 6, 7]]


def _slopes():
    i = np.arange(1, H + 1, dtype=np.float64)
    return 1.0 / np.power(2.0, 8.0 * i / H)


# ----------------------------------------------------------------------------
# device program (identical on all 8 cores; per-core behavior via input data)
# ----------------------------------------------------------------------------
def _build(has_b1):
    K9 = 2 if has_b1 else 1
    nc = bass.Bass("TRN2", num_devices=N_CORES)

    xT_d = nc.dram_tensor("xT", [128, CCH, T], BF16, kind="ExternalInput")
    xtok_d = nc.dram_tensor("xtok", [T, C], BF16, kind="ExternalInput")
    xr_d = nc.dram_tensor("x_rows", [ROWS, C], F32, kind="ExternalInput")
    wq_d = nc.dram_tensor("wq", [128, 2, CCH, 128], BF16, kind="ExternalInput")
    wk_d = nc.dram_tensor("wk", [128, 2, CCH, 128], BF16, kind="ExternalInput")
    wv_d = nc.dram_tensor("wv", [128, CCH, 256], BF16, kind="ExternalInput")
    wq9_d = nc.dram_tensor("wq9", [2, 2, 128], BF16, kind="ExternalInput")
    wk9_d = nc.dram_tensor("wk9", [2, 2, 128], BF16, kind="ExternalInput")
    wv9_d = nc.dram_tensor("wv9", [2, 256], BF16, kind="ExternalInput")
    wp_d = nc.dram_tensor("wproj", [128, 2, 1024], BF16, kind="ExternalInput")
    wg_d = nc.dram_tensor("wg", [128, MCH, CCH, 128], BF16, kind="ExternalInput")
    wu_d = nc.dram_tensor("wu", [128, MCH, CCH, 128], BF16, kind="ExternalInput")
    wd_d = nc.dram_tensor("wd", [128, MCH, 1024], BF16, kind="ExternalInput")
    bg_d = nc.dram_tensor("bg", [128, MCH], F32, kind="ExternalInput")
    bu_d = nc.dram_tensor("bu", [128, MCH], F32, kind="ExternalInput")
    db_d = nc.dram_tensor("dbias", [128, 1024], F32, kind="ExternalInput")
    mk_d = nc.dram_tensor("masks", [128, 4, 1024], BF16, kind="ExternalInput")
    ckb_d = nc.dram_tensor("ckb", [128, NTT, HL], BF16, kind="ExternalInput")
    ckf_d = nc.dram_tensor("ckf", [128, NTT, HL], F32, kind="ExternalInput")
    sel2_d = nc.dram_tensor("sel2", [128, 128], F32, kind="ExternalInput")

    out_d = nc.dram_tensor("out_rows", [ROWS, C], F32, kind="ExternalOutput")

    from contextlib import ExitStack
    with tile.TileContext(nc) as tc, ExitStack() as top:
        consts = top.enter_context(tc.tile_pool(name="consts", bufs=1))
        stats = top.enter_context(tc.tile_pool(name="stats", bufs=16))
        work = top.enter_context(tc.tile_pool(name="work", bufs=3))
        dramp = top.enter_context(tc.tile_pool(name="dram", bufs=1, space="DRAM"))

        eps_ap = consts.tile([128, 1], F32)
        nc.vector.memset(eps_ap[:], EPS)
        cb24 = consts.tile([128, 1], F32)
        nc.vector.memset(cb24[:], -CBIAS)
        ones1 = consts.tile([1, 128], F32)
        nc.vector.memset(ones1[:], 1.0)
        sel2 = consts.tile([128, 128], F32)
        nc.gpsimd.dma_start(sel2[:], sel2_d[:, :])
        ck32 = consts.tile([128, NTT, HL], F32)
        nc.gpsimd.dma_start(ck32[:], ckf_d[:, :, :])
        bg_sb = consts.tile([128, MCH], F32)
        nc.gpsimd.dma_start(bg_sb[:], bg_d[:, :])
        bu_sb = consts.tile([128, MCH], F32)
        nc.gpsimd.dma_start(bu_sb[:], bu_d[:, :])
        ident = consts.tile([128, 128], BF16)
        from concourse.masks import make_identity
        make_identity(nc, ident[:])

        # persistent cross-phase state
        x2 = consts.tile([128, RT, C], F32)
        y2T = consts.tile([128, CCH, ROWS], BF16)
        mv2 = [consts.tile([128, 2], F32, name=f"mv2_{r}") for r in range(RT)]

        rs_in = [[dramp.tile([512, 512], BF16, name=f"rsin{i}_{nb}")
                  for nb in range(2)] for i in range(NTB)]
        rs_out = [[dramp.tile([128, 512], BF16, name=f"rsout{i}_{nb}")
                   for nb in range(2)] for i in range(NTB)]

        with ExitStack() as attn_scope:
            apool = attn_scope.enter_context(tc.tile_pool(name="attn", bufs=1))
            pipe = attn_scope.enter_context(tc.tile_pool(name="pipe", bufs=4))
            OnTp = attn_scope.enter_context(tc.tile_pool(name="OnTp", bufs=2))
            recp = attn_scope.enter_context(tc.tile_pool(name="recp", bufs=2))

            wq_sb = apool.tile([128, 2, CCH, 128], BF16)
            nc.gpsimd.dma_start(wq_sb[:], wq_d[:, :, :, :])
            wk_sb = apool.tile([128, 2, CCH, 128], BF16)
            nc.gpsimd.dma_start(wk_sb[:], wk_d[:, :, :, :])
            wv_sb = apool.tile([128, CCH, 256], BF16)
            nc.gpsimd.dma_start(wv_sb[:], wv_d[:, :, :])
            wq9_sb = apool.tile([2, 2, 128], BF16)
            nc.gpsimd.dma_start(wq9_sb[:], wq9_d[:, :, :])
            wk9_sb = apool.tile([2, 2, 128], BF16)
            nc.gpsimd.dma_start(wk9_sb[:], wk9_d[:, :, :])
            wv9_sb = apool.tile([2, 256], BF16)
            nc.gpsimd.dma_start(wv9_sb[:], wv9_d[:, :])
            wp_sb = apool.tile([128, 2, 1024], BF16)
            nc.gpsimd.dma_start(wp_sb[:], wp_d[:, :, :])
            masks = apool.tile([128, 4, 1024], BF16)
            nc.gpsimd.dma_start(masks[:], mk_d[:, :, :])

            qT2 = apool.tile([128, 2, T], BF16)
            kT2 = apool.tile([128, 2, T], BF16)
            Vh = apool.tile([128, NTT, HL, 66], BF16)
            nc.gpsimd.memset(Vh[:, :, :, 65:66], 0.0)
            nc.gpsimd.dma_start(Vh[:, :, :, 64:65], ckb_d[:, :, :])

            dpad2s = []
            for i in range(2):
                dp = apool.tile([128, 512], F32, name=f"dpad{i}")
                nc.gpsimd.memset(dp[:], 0.0)
                dpad2s.append(dp)

            with ExitStack() as qkv_scope:
                qpool = qkv_scope.enter_context(tc.tile_pool(name="qkv", bufs=1))
                psQ = qkv_scope.enter_context(
                    tc.tile_pool(name="psQ", bufs=4, space="PSUM"))
                psB = qkv_scope.enter_context(
                    tc.tile_pool(name="psB", bufs=1, space="PSUM"))
                psV = qkv_scope.enter_context(
                    tc.tile_pool(name="psV", bufs=2, space="PSUM"))

                xT_sb = qpool.tile([128, CCH, T], BF16)
                for cc in range(CCH):
                    nc.sync.dma_start(xT_sb[:, cc, :], xT_d[:, cc, :])

                mrow = qpool.tile([2, T], BF16)    # row0: -mu, row1: ones(b1)
                nc.vector.memset(mrow[:], 0.0)
                if has_b1:
                    nc.vector.memset(mrow[1:2, :], 1.0)
                rstd_row = qpool.tile([1, T], F32)
                rstdb = [qpool.tile([128, 512], F32, name=f"rstdb{b}")
                         for b in range(NTB)]
                rstds = [qpool.tile([128, 1], F32, name=f"rstds{t}")
                         for t in range(NTT)]

                # ---- per-token-tile LN stats (vector) --------------------
                mucB = [qpool.tile([128, 32], F32, name=f"mucB{b}")
                        for b in range(NTB)]
                for b in range(NTB):
                    nc.gpsimd.memset(mucB[b][:], 0.0)
                for t in range(NTT):
                    xt = work.tile([128, C], BF16, tag="xt")
                    nc.scalar.dma_start(xt[:], xtok_d[ts(t, 128), :])
                    bst = stats.tile([128, 2, 6], F32, tag="bst")
                    for sg in range(2):
                        nc.vector.bn_stats(bst[:, sg, :], xt[:, ts(sg, 512)])
                    mv = stats.tile([128, 2], F32, tag="mv")
                    nc.vector.bn_aggr(mv[:], bst[:])
                    std = stats.tile([128, 1], F32, tag="std")
                    nc.scalar.activation(std[:], mv[:, 1:2],
                                         mybir.ActivationFunctionType.Sqrt,
                                         bias=eps_ap[:])
                    nc.vector.reciprocal(rstds[t][:], std[:])
                    b, i = t // 4, t % 4
                    nc.vector.tensor_scalar_mul(mucB[b][:, i:i + 1],
                                                mv[:, 0:1], -1.0)
                    nc.vector.tensor_copy(mucB[b][:, 4 + i:5 + i], rstds[t][:])

                for b in range(NTB):
                    stt = stats.tile([128, 32], F32, tag="stt")
                    nc.vector.transpose(stt[:], mucB[b][:])
                    sTb = stats.tile([8, 128], BF16, tag="sTb")
                    sTf = stats.tile([8, 128], F32, tag="sTf")
                    for i in range(4):
                        nc.vector.tensor_copy(sTb[0:8, ts(i, 32)],
                                              stt[32 * i:32 * i + 8, :])
                        nc.vector.tensor_copy(sTf[0:8, ts(i, 32)],
                                              stt[32 * i:32 * i + 8, :])
                    for i in range(4):
                        nc.gpsimd.dma_start(
                            mrow[0:1, 512 * b + 128 * i:512 * b + 128 * (i + 1)],
                            sTb[i:i + 1, :])
                        nc.gpsimd.dma_start(
                            rstd_row[0:1, 512 * b + 128 * i:512 * b + 128 * (i + 1)],
                            sTf[4 + i:5 + i, :])

                # ---- QKV -------------------------------------------------
                for b in range(NTB):
                    pss = {}
                    for p in range(2):
                        for wi in range(2):
                            pss[(p, wi)] = psQ.tile(
                                [128, 512], F32, tag="qk",
                                name=f"qk_{b}_{p}_{wi}")
                    for cc in range(CCH):
                        for p in range(2):
                            for wi, wsb in enumerate((wq_sb, wk_sb)):
                                nc.tensor.matmul(
                                    pss[(p, wi)][:],
                                    wsb[:, p, cc, :],
                                    xT_sb[:, cc, ts(b, 512)],
                                    start=(cc == 0), stop=False)
                    for p in range(2):
                        for wi, w9 in enumerate((wq9_sb, wk9_sb)):
                            nc.tensor.matmul(
                                pss[(p, wi)][:], w9[0:K9, p, :],
                                mrow[0:K9, ts(b, 512)],
                                start=False, stop=True)
                    psb = psB.tile([128, 512], F32, tag="psb")
                    nc.tensor.matmul(psb[:], ones1[0:1, :],
                                     rstd_row[0:1, ts(b, 512)],
                                     start=True, stop=True)
                    nc.scalar.copy(rstdb[b][:], psb[:])
                    for p in range(2):
                        for wi, dstT in enumerate((qT2, kT2)):
                            nc.vector.tensor_tensor(
                                dstT[:, p, ts(b, 512)],
                                pss[(p, wi)][:], rstdb[b][:],
                                mybir.AluOpType.mult)

                for t in range(NTT):
                    psv = psV.tile([128, 256], F32, tag="v")
                    for cc in range(CCH):
                        nc.tensor.matmul(psv[:], xT_sb[:, cc, ts(t, 128)],
                                         wv_sb[:, cc, :],
                                         start=(cc == 0), stop=False)
                    nc.tensor.matmul(psv[:], mrow[0:K9, ts(t, 128)],
                                     wv9_sb[0:K9, :], start=False, stop=True)
                    for h in range(HL):
                        nc.vector.tensor_scalar(
                            Vh[:, t, h, 0:64], psv[:, ts(h, 64)],
                            rstds[t][:], ck32[:, t, h:h + 1],
                            mybir.AluOpType.mult, mybir.AluOpType.mult)

            # ---- attention + deferred proj/LN2 + pipelined AllToAll -----
            with ExitStack() as ps_scope:
                psS = ps_scope.enter_context(
                    tc.tile_pool(name="psS", bufs=3, space="PSUM"))
                psO = ps_scope.enter_context(
                    tc.tile_pool(name="psO", bufs=2, space="PSUM"))

                def emit_proj(c):
                    # proj partial over local heads -> DRAM, RS per col-half
                    for nb in range(2):
                        for i2 in range(4):
                            pp = psS.tile([128, 512], F32, tag="pp", bufs=2,
                                          name=f"pp_{c}_{nb}_{i2}")
                            for cc in range(2):
                                nc.tensor.matmul(pp[:],
                                                 OnT_cs[c][:, cc, ts(i2, 128)],
                                                 wp_sb[:, cc, ts(nb, 512)],
                                                 start=(cc == 0), stop=(cc == 1))
                            pjs = pipe.tile([128, 512], BF16, tag="pjs",
                                            bufs=2, name=f"pjs_{c}_{nb}_{i2}")
                            nc.scalar.copy(pjs[:], pp[:])
                            nc.sync.dma_start(rs_in[c][nb][ts(i2, 128), :],
                                              pjs[:])
                        nc.gpsimd.collective_compute(
                            "ReduceScatter", mybir.AluOpType.add,
                            replica_groups=GROUPS,
                            ins=[rs_in[c][nb].opt()],
                            outs=[rs_out[c][nb].opt()],
                        )

                def emit_ln2(c):
                    xrt = work.tile([128, C], F32, tag="xrt", bufs=2)
                    nc.sync.dma_start(xrt[:], xr_d[ts(c, 128), :])
                    for nb in range(2):
                        rst = work.tile([128, 512], BF16, tag="rst", bufs=2)
                        nc.sync.dma_start(rst[:], rs_out[c][nb][:, :])
                        nc.vector.tensor_tensor(x2[:, c, ts(nb, 512)], rst[:],
                                                xrt[:, ts(nb, 512)],
                                                mybir.AluOpType.add)
                    bst = stats.tile([128, 2, 6], F32, tag="bst2")
                    for sg in range(2):
                        nc.vector.bn_stats(bst[:, sg, :], x2[:, c, ts(sg, 512)])
                    nc.vector.bn_aggr(mv2[c][:], bst[:])
                    std = stats.tile([128, 1], F32, tag="std2")
                    nc.scalar.activation(std[:], mv2[c][:, 1:2],
                                         mybir.ActivationFunctionType.Sqrt,
                                         bias=eps_ap[:])
                    rstd2 = stats.tile([128, 1], F32, tag="rstd2")
                    nc.vector.reciprocal(rstd2[:], std[:])
                    yb = work.tile([128, C], BF16, tag="yb", bufs=2)
                    nc.vector.tensor_scalar(yb[:], x2[:, c, :], mv2[c][:, 0:1],
                                            rstd2[:], mybir.AluOpType.subtract,
                                            mybir.AluOpType.mult)
                    for half in range(2):
                        pt = psS.tile([128, 4, 128], BF16, tag="pp", bufs=2,
                                      name=f"tr2_{c}_{half}")
                        for i in range(4):
                            cc = half * 4 + i
                            nc.tensor.transpose(pt[:, i, :], yb[:, ts(cc, 128)],
                                                ident[:])
                        nc.vector.tensor_copy(
                            y2T[:, half * 4:(half + 1) * 4, ts(c, 128)], pt[:])

                npair = [0]
                prev_c = None
                OnT_cs = {}
                for c in (3, 2, 1, 0):
                    OnT_c = OnTp.tile([128, 2, 512], BF16, tag="OnT")
                    OnT_cs[c] = OnT_c
                    t_hi = 4 * c + 4
                    for p in range(2):
                        po0 = psO.tile([66, 512], F32, tag="po",
                                       name=f"po0_{c}_{p}")
                        po1 = psO.tile([66, 512], F32, tag="po",
                                       name=f"po1_{c}_{p}")

                        def emit_s(t):
                            if t >= 4 * c:
                                v = t - 4 * c
                                w = 512 - 128 * v
                            else:
                                v, w = -1, 512
                            q0 = 512 * c + 512 - w
                            st0 = psS.tile([128, 512], F32, tag="s",
                                           name=f"s0_{c}_{p}_{t}")
                            st1 = psS.tile([128, 512], F32, tag="s",
                                           name=f"s1_{c}_{p}_{t}")
                            nc.tensor.matmul(st0[:, 0:w],
                                             kT2[0:64, p, ts(t, 128)],
                                             qT2[0:64, p, q0:q0 + w],
                                             start=True, stop=True)
                            nc.tensor.matmul(st1[:, 0:w],
                                             kT2[64:128, p, ts(t, 128)],
                                             qT2[64:128, p, q0:q0 + w],
                                             start=True, stop=True)
                            pT = pipe.tile([128, 1024], BF16, tag="pT",
                                           name=f"pT_{c}_{p}_{t}")
                            nc.scalar.activation(
                                pT[:, 0:w], st0[:, 0:w],
                                mybir.ActivationFunctionType.Exp, bias=cb24[:])
                            nc.scalar.activation(
                                pT[:, w:2 * w], st1[:, 0:w],
                                mybir.ActivationFunctionType.Exp, bias=cb24[:])
                            if v >= 0:
                                nc.vector.tensor_tensor(
                                    pT[:, 0:2 * w], pT[:, 0:2 * w],
                                    masks[:, v, 0:2 * w], mybir.AluOpType.mult)
                            return pT, w

                        pTs = {0: emit_s(0)}
                        if t_hi > 1:
                            pTs[1] = emit_s(1)
                        for t in range(t_hi):
                            pT, w = pTs.pop(t)
                            nc.tensor.matmul(po0[:, 512 - w:512],
                                             Vh[:, t, 2 * p, 0:66],
                                             pT[:, 0:w], start=(t == 0),
                                             stop=(t == t_hi - 1),
                                             skip_group_check=True)
                            nc.tensor.matmul(po1[:, 512 - w:512],
                                             Vh[:, t, 2 * p + 1, 0:66],
                                             pT[:, w:2 * w], start=(t == 0),
                                             stop=(t == t_hi - 1),
                                             skip_group_check=True)
                            if t + 2 < t_hi:
                                pTs[t + 2] = emit_s(t + 2)
                        # softmax denominators for the pair -> broadcast
                        dp = dpad2s[npair[0] % 2]
                        npair[0] += 1
                        nc.vector.tensor_copy(dp[0:1, :], po0[64:65, :])
                        nc.vector.tensor_copy(dp[64:65, :], po1[64:65, :])
                        rb = psS.tile([128, 512], F32, tag="rb", bufs=1,
                                      name=f"rb_{c}_{p}")
                        nc.tensor.matmul(rb[:], sel2[:], dp[:],
                                         start=True, stop=True)
                        rec = recp.tile([128, 512], F32, tag="rec")
                        nc.vector.reciprocal(rec[:], rb[:])
                        nc.vector.tensor_tensor(OnT_c[0:64, p, :], po0[0:64, :],
                                                rec[0:64, :],
                                                mybir.AluOpType.mult)
                        nc.vector.tensor_tensor(OnT_c[64:128, p, :],
                                                po1[0:64, :], rec[64:128, :],
                                                mybir.AluOpType.mult)
                    emit_proj(c)
                    if prev_c is not None:
                        emit_ln2(prev_c)
                    prev_c = c
                emit_ln2(0)

        # ---- SwiGLU MLP (row-parallel, full weights) --------------------
        with ExitStack() as mlp_scope:
            mpool = mlp_scope.enter_context(tc.tile_pool(name="mlp", bufs=1))
            wstream = mlp_scope.enter_context(tc.tile_pool(name="wstream", bufs=3))
            psC = mlp_scope.enter_context(
                tc.tile_pool(name="psC", bufs=2, space="PSUM"))

            gu = mpool.tile([128, MCH, ROWS], BF16)
            db_sb = mpool.tile([128, 1024], F32)
            nc.gpsimd.dma_start(db_sb[:], db_d[:, :])
            wd_sb = mpool.tile([128, MCH, 1024], BF16)
            for half in range(2):
                nc.gpsimd.dma_start(wd_sb[:, :, ts(half, 512)],
                                    wd_d[:, :, ts(half, 512)])

            for hc in range(MCH):
                wgt = wstream.tile([128, CCH, 128], BF16, tag="wgt")
                nc.sync.dma_start(wgt[:], wg_d[:, hc, :, :])
                wut = wstream.tile([128, CCH, 128], BF16, tag="wut")
                nc.sync.dma_start(wut[:], wu_d[:, hc, :, :])
                pg = psC.tile([128, 512], F32, tag="g")
                pu = psC.tile([128, 512], F32, tag="u")
                for cc in range(CCH):
                    nc.tensor.matmul(pg[:], wgt[:, cc, :], y2T[:, cc, :],
                                     start=(cc == 0), stop=(cc == CCH - 1))
                for cc in range(CCH):
                    nc.tensor.matmul(pu[:], wut[:, cc, :], y2T[:, cc, :],
                                     start=(cc == 0), stop=(cc == CCH - 1))
                gs = work.tile([128, 512], BF16, tag="gs")
                nc.scalar.activation(gs[:], pg[:],
                                     mybir.ActivationFunctionType.Silu,
                                     bias=bg_sb[:, hc:hc + 1])
                us = work.tile([128, 512], BF16, tag="us")
                nc.scalar.activation(us[:], pu[:],
                                     mybir.ActivationFunctionType.Identity,
                                     bias=bu_sb[:, hc:hc + 1])
                nc.vector.tensor_tensor(gu[:, hc, :], gs[:], us[:],
                                        mybir.AluOpType.mult)

            for nb in range(2):
                for tt in range(RT):
                    pd = psC.tile([128, 512], F32, tag="d")
                    for hc in range(MCH):
                        nc.tensor.matmul(pd[:], gu[:, hc, ts(tt, 128)],
                                         wd_sb[:, hc, ts(nb, 512)],
                                         start=(hc == 0), stop=(hc == MCH - 1))
                    o1 = work.tile([128, 512], F32, tag="o1")
                    nc.vector.tensor_tensor(o1[:], pd[:],
                                            x2[:, tt, ts(nb, 512)],
                                            mybir.AluOpType.add)
                    nc.vector.tensor_tensor(o1[:], o1[:],
                                            db_sb[:, ts(nb, 512)],
                                            mybir.AluOpType.add)
                    nc.sync.dma_start(out_d[ts(tt, 128), ts(nb, 512)], o1[:])

    _split_excess_waits(nc)
    return nc


# ----------------------------------------------------------------------------
# host-side input prep + launch
# ----------------------------------------------------------------------------
_cache = {}

def _get_nc(has_b1):
    if has_b1 not in _cache:
        _cache[has_b1] = _build(has_b1)
    return _cache[has_b1]


def _prep(x, ln1_g, ln1_b, qkv_w, qkv_b, proj_w, proj_b,
          ln2_g, ln2_b, gate_w, gate_b, up_w, up_b, down_w, down_b):
    x = np.asarray(x, np.float32)
    f = lambda a: np.asarray(a, np.float32)
    ln1_g, ln1_b, qkv_b, proj_b, ln2_g, ln2_b = map(f, (
        ln1_g, ln1_b, qkv_b, proj_b, ln2_g, ln2_b))
    qkv_w, proj_w, gate_w, gate_b, up_w, up_b, down_w, down_b = map(f, (
        qkv_w, proj_w, gate_w, gate_b, up_w, up_b, down_w, down_b))

    slopes = _slopes()

    # fold LN affines into the consuming matmuls
    w1 = qkv_w * ln1_g[:, None]
    b1 = ln1_b @ qkv_w + qkv_b              # [3C]
    wg_f = gate_w * ln2_g[:, None]
    bg_f = ln2_b @ gate_w + gate_b          # [HID]
    wu_f = up_w * ln2_g[:, None]
    bu_f = ln2_b @ up_w + up_b

    has_b1 = bool(np.any(b1 != 0.0))

    wgp = np.zeros((C, HIDP), np.float32); wgp[:, :HID] = wg_f
    wup = np.zeros((C, HIDP), np.float32); wup[:, :HID] = wu_f
    wdp = np.zeros((HIDP, 1024), np.float32); wdp[:HID] = down_w
    bgp = np.zeros(HIDP, np.float32); bgp[:HID] = bg_f
    bup = np.zeros(HIDP, np.float32); bup[:HID] = bu_f

    wg_dev = wgp.reshape(CCH, 128, MCH, 128).transpose(1, 2, 0, 3).astype(bf16)
    wu_dev = wup.reshape(CCH, 128, MCH, 128).transpose(1, 2, 0, 3).astype(bf16)
    wd_dev = wdp.reshape(MCH, 128, 1024).transpose(1, 0, 2).astype(bf16)
    bg_dev = bgp.reshape(MCH, 128).T.copy()
    bu_dev = bup.reshape(MCH, 128).T.copy()
    db_dev = np.broadcast_to(down_b, (128, 1024)).copy()

    # triangular diagonal masks, trimmed + packed for head pairs
    pp_i = np.arange(128)[:, None]
    jj = np.arange(512)[None, :]
    tri = (jj >= pp_i).astype(bf16)          # [128, 512]
    masks_np = np.zeros((128, 4, 1024), bf16)
    for v in range(4):
        w = 512 - 128 * v
        masks_np[:, v, 0:w] = tri[:, 0:w]
        masks_np[:, v, w:2 * w] = tri[:, 0:w]

    sel2_np = np.zeros((128, 128), np.float32)
    sel2_np[0, 0:64] = 1.0
    sel2_np[64, 64:128] = 1.0

    def qkv_w9(wcols, bcols):
        w9 = np.zeros((2, 2, 128), np.float32)
        w9[0] = wcols.sum(axis=0).reshape(2, 128)
        w9[1] = bcols.reshape(2, 128)
        return w9.astype(bf16)

    in_maps = []
    for core in range(N_CORES):
        b, g = core // TP, core % TP
        heads = range(4 * g, 4 * g + 4)
        qcols = np.concatenate([np.arange(h * D, (h + 1) * D) for h in heads])
        kcols = qcols + C
        vcols = qcols + 2 * C

        wq_cols = w1[:, qcols] * 0.125           # [C, 256]
        wk_cols = w1[:, kcols]
        wv_cols = w1[:, vcols]
        wq_dev = wq_cols.reshape(CCH, 128, 2, 128).transpose(1, 2, 0, 3).astype(bf16)
        wk_dev = wk_cols.reshape(CCH, 128, 2, 128).transpose(1, 2, 0, 3).astype(bf16)
        wv_dev = wv_cols.reshape(CCH, 128, 256).transpose(1, 0, 2).astype(bf16)
        wq9_dev = qkv_w9(wq_cols, b1[qcols] * 0.125)
        wk9_dev = qkv_w9(wk_cols, b1[kcols])
        wv9_dev = np.zeros((2, 256), np.float32)
        wv9_dev[0] = wv_cols.sum(axis=0)
        wv9_dev[1] = b1[vcols]
        wv9_dev = wv9_dev.astype(bf16)

        wp_rows = proj_w[qcols, :]                        # [256, 1024]
        wp_dev = wp_rows.reshape(2, 128, 1024).transpose(1, 0, 2).astype(bf16)

        # ALiBi key-side factors folded into V (and the denominator column)
        ck = np.zeros((128, NTT, HL), np.float64)
        for hl, h in enumerate(heads):
            sl = slopes[h]
            for t in range(NTT):
                ck[:, t, hl] = np.exp(-sl * (128 * t + np.arange(128)))
        ckf = ck.astype(np.float32)

        xb = x[b]                                # [T, C]
        xT_dev = np.ascontiguousarray(
            xb.T.reshape(CCH, 128, T).transpose(1, 0, 2)).astype(bf16)

        in_maps.append({
            "xT": xT_dev,
            "xtok": xb.astype(bf16),
            "x_rows": np.concatenate(
                [xb[512 * r + 128 * g:512 * r + 128 * g + 128]
                 for r in range(4)], axis=0) + proj_b[None, :],
            "wq": wq_dev, "wk": wk_dev, "wv": wv_dev,
            "wq9": wq9_dev, "wk9": wk9_dev, "wv9": wv9_dev,
            "wproj": wp_dev,
            "wg": wg_dev, "wu": wu_dev, "wd": wd_dev,
            "bg": bg_dev, "bu": bu_dev, "dbias": db_dev,
            "masks": masks_np, "ckb": ckf.astype(bf16), "ckf": ckf,
            "sel2": sel2_np,
        })

    return has_b1, in_maps


def _gather(results):
    out = np.empty((B, T, C), np.float32)
    for core in range(N_CORES):
        b, g = core // TP, core % TP
        orr = results[core]["out_rows"]
        for r in range(4):
            out[b, 512 * r + 128 * g:512 * r + 128 * g + 128] = \
                orr[128 * r:128 * (r + 1)]
    return out


def kernel(**inputs):
    has_b1, in_maps = _prep(**inputs)
    nc = _get_nc(has_b1)
    res = bass_utils.run_bass_kernel_spmd(
        nc, in_maps, core_ids=list(range(N_CORES)))
    return _gather(res.results)


# revision 22
# speedup vs baseline: 1.1249x; 1.1249x over previous
"""Trainium2 Bass kernel for EnhancedTransformerBlock (B=2,T=2048,C=1024,H=16,
SwiGLU HIDDEN=2730, ALiBi-abs + causal attention).

Sharding over 8 cores: batch (2) x head-groups (4 heads/core). Host ships x
both token-major (bf16, LN stats only) and feature-major (bf16, matmul
stream); LN1 is folded into the QKV matmuls via a mean-row augmentation plus
a per-token rstd column scale applied on the PSUM->SBUF copy. Attention runs
with transposed scores S^T[tk,tq]; the ALiBi key-side bias is folded into
V's rows (so the exp bias is a shared constant), the softmax denominator
rides as a scaled ones column through the PV matmul, and causally-dead query
columns are trimmed from the diagonal S/exp/PV tiles. proj partials are
ReduceScattered per 512-token block (bf16); the SwiGLU MLP runs row-parallel
with fp8(e4m3) DoubleRow matmuls. Host reassembles row blocks.
"""
import sys, types
sys.path.insert(0, "/opt/trn_rl_repo")
import numpy as np
import ml_dtypes

import concourse.bass as bass
import concourse.tile as tile
from concourse import mybir
import concourse.bass_utils as bass_utils
import bass_rust

# ----------------------------------------------------------------------------
# environment patches (walrus in this container accepts only 1 sync-wait/inst)
# ----------------------------------------------------------------------------
_DRAIN_WAIT_LIMIT = 1

def _patched_drain_and_barrier(self, tick_clock, wait_clock):
    nc = self.nc
    drain_inst = nc.sync.drain()
    wait_clock.add_sem_waits(
        drain_inst.ins, bass_rust.ScopedClock({None: tick_clock.global_clock})
    )
    si = drain_inst.ins.sync_info
    waits = list(si.on_wait) if si is not None else []
    if len(waits) > _DRAIN_WAIT_LIMIT:
        si.on_wait = waits[:_DRAIN_WAIT_LIMIT]
        for i in range(_DRAIN_WAIT_LIMIT, len(waits), _DRAIN_WAIT_LIMIT):
            d2 = nc.sync.drain()
            d2.ins.sync_info = bass_rust.SyncInfo(
                on_wait=waits[i:i + _DRAIN_WAIT_LIMIT], on_update=[]
            )
    nc.all_engine_barrier()
    popped = nc._tile_sem_poison_stack.pop()
    assert popped is self._sem_poison
    nc.clear_and_free_semaphores(list(self.sems.allocated().values()))
    nc.all_engine_barrier()


def _split_excess_waits(nc, limit=_DRAIN_WAIT_LIMIT):
    n = [0]
    for bb in nc.main_func.blocks:
        insts = bb.instructions
        out = []
        changed = False
        for inst in insts:
            si = inst.sync_info
            waits = list(si.on_wait) if si is not None else []
            if len(waits) > limit:
                changed = True
                keep = waits[-limit:]
                rest = waits[:-limit]
                for i in range(0, len(rest), limit):
                    n[0] += 1
                    d = mybir.InstNoOp(
                        name=f"waitsplit-{n[0]}", engine=inst.engine, ins=[], outs=[]
                    )
                    d.sync_info = bass_rust.SyncInfo(
                        on_wait=rest[i:i + limit], on_update=[]
                    )
                    out.append(d)
                si.on_wait = keep
            out.append(inst)
        if changed:
            bb.instructions = out


def _install_patches():
    tile.TileContext._drain_and_barrier = _patched_drain_and_barrier
    if "antenv.axon_hooks" not in sys.modules:
        try:
            from trn_agent_boot.trn_boot import _ntff_profile_via_ctypes
            hook = _ntff_profile_via_ctypes("/opt/axon/libaxon_pjrt.so")
        except Exception:
            hook = None
        mod = types.ModuleType("antenv.axon_hooks")
        mod.get_axon_ntff_profile_hook = lambda: hook
        mod.set_axon_ntff_profile_hook = lambda h: None
        sys.modules["antenv.axon_hooks"] = mod
        bass_utils.upload_artifacts = lambda tmpdir: tmpdir

_install_patches()

# ----------------------------------------------------------------------------
# problem constants
# ----------------------------------------------------------------------------
B, T, C = 2, 2048, 1024
H, D = 16, 64
HID, HIDP = 2730, 2816
N_CORES, TP = 8, 4
ROWS = T // TP          # 512 rows per core after the reduce-scatter
HL = 4                  # local heads per core
EPS = 1e-5
CBIAS = 24.0
F32, BF16 = mybir.dt.float32, mybir.dt.bfloat16
F8 = mybir.dt.float8e4
bf16 = ml_dtypes.bfloat16
f8np = ml_dtypes.float8_e4m3
ts = bass.ts

NTT = T // 128          # 16 token tiles
NTB = T // 512          # 4 token blocks
CCH = C // 128          # 8 feature chunks
MCH = HIDP // 128       # 22 hidden chunks
RT = ROWS // 128        # 4 row tiles per core
GROUPS = [[0, 1, 2, 3], [4, 5, 6, 7]]
DR = mybir.MatmulPerfMode.DoubleRow


def _slopes():
    i = np.arange(1, H + 1, dtype=np.float64)
    return 1.0 / np.power(2.0, 8.0 * i / H)


# ----------------------------------------------------------------------------
# device program (identical on all 8 cores; per-core behavior via input data)
# ----------------------------------------------------------------------------
def _build(has_b1):
    K9 = 2 if has_b1 else 1
    nc = bass.Bass("TRN2", num_devices=N_CORES)

    xT_d = nc.dram_tensor("xT", [128, CCH, T], BF16, kind="ExternalInput")
    xtok_d = nc.dram_tensor("xtok", [T, C], BF16, kind="ExternalInput")
    xr_d = nc.dram_tensor("x_rows", [ROWS, C], F32, kind="ExternalInput")
    wq_d = nc.dram_tensor("wq", [128, 2, CCH, 128], BF16, kind="ExternalInput")
    wk_d = nc.dram_tensor("wk", [128, 2, CCH, 128], BF16, kind="ExternalInput")
    wv_d = nc.dram_tensor("wv", [128, CCH, 256], BF16, kind="ExternalInput")
    wq9_d = nc.dram_tensor("wq9", [2, 2, 128], BF16, kind="ExternalInput")
    wk9_d = nc.dram_tensor("wk9", [2, 2, 128], BF16, kind="ExternalInput")
    wv9_d = nc.dram_tensor("wv9", [2, 256], BF16, kind="ExternalInput")
    wp_d = nc.dram_tensor("wproj", [128, 2, 1024], BF16, kind="ExternalInput")
    wg_d = nc.dram_tensor("wg", [128, MCH, CCH, 128], F8, kind="ExternalInput")
    wu_d = nc.dram_tensor("wu", [128, MCH, CCH, 128], F8, kind="ExternalInput")
    wd_d = nc.dram_tensor("wd", [128, MCH, 1024], F8, kind="ExternalInput")
    bg_d = nc.dram_tensor("bg", [128, MCH], F32, kind="ExternalInput")
    bu_d = nc.dram_tensor("bu", [128, MCH], F32, kind="ExternalInput")
    db_d = nc.dram_tensor("dbias", [128, 1024], F32, kind="ExternalInput")
    mk_d = nc.dram_tensor("masks", [128, 512], BF16, kind="ExternalInput")
    ckb_d = nc.dram_tensor("ckb", [128, NTT, HL], BF16, kind="ExternalInput")
    ckf_d = nc.dram_tensor("ckf", [128, NTT, HL], F32, kind="ExternalInput")
    sel2_d = nc.dram_tensor("sel2", [128, 128], F32, kind="ExternalInput")

    out_d = nc.dram_tensor("out_rows", [ROWS, C], F32, kind="ExternalOutput")

    from contextlib import ExitStack
    with tile.TileContext(nc) as tc, ExitStack() as top:
        consts = top.enter_context(tc.tile_pool(name="consts", bufs=1))
        stats = top.enter_context(tc.tile_pool(name="stats", bufs=16))
        work = top.enter_context(tc.tile_pool(name="work", bufs=3))
        dramp = top.enter_context(tc.tile_pool(name="dram", bufs=1, space="DRAM"))

        eps_ap = consts.tile([128, 1], F32)
        nc.vector.memset(eps_ap[:], EPS)
        cb24 = consts.tile([128, 1], F32)
        nc.vector.memset(cb24[:], -CBIAS)
        ones1 = consts.tile([1, 128], F32)
        nc.vector.memset(ones1[:], 1.0)
        sel2 = consts.tile([128, 128], F32)
        nc.gpsimd.dma_start(sel2[:], sel2_d[:, :])
        ck32 = consts.tile([128, NTT, HL], F32)
        nc.gpsimd.dma_start(ck32[:], ckf_d[:, :, :])
        bg_sb = consts.tile([128, MCH], F32)
        nc.gpsimd.dma_start(bg_sb[:], bg_d[:, :])
        bu_sb = consts.tile([128, MCH], F32)
        nc.gpsimd.dma_start(bu_sb[:], bu_d[:, :])
        ident = consts.tile([128, 128], BF16)
        from concourse.masks import make_identity
        make_identity(nc, ident[:])

        # persistent cross-phase state
        x2 = consts.tile([128, RT, C], F32)
        y2T8 = consts.tile([128, CCH, ROWS], F8)
        mv2 = [consts.tile([128, 2], F32, name=f"mv2_{r}") for r in range(RT)]

        rs_in = [dramp.tile([512, C], BF16, name=f"rsin{i}") for i in range(NTB)]
        rs_out = [dramp.tile([128, C], BF16, name=f"rsout{i}")
                  for i in range(NTB)]

        with ExitStack() as attn_scope:
            apool = attn_scope.enter_context(tc.tile_pool(name="attn", bufs=1))
            pipe = attn_scope.enter_context(tc.tile_pool(name="pipe", bufs=4))
            OnTp = attn_scope.enter_context(tc.tile_pool(name="OnTp", bufs=2))
            recp = attn_scope.enter_context(tc.tile_pool(name="recp", bufs=2))

            wq_sb = apool.tile([128, 2, CCH, 128], BF16)
            nc.gpsimd.dma_start(wq_sb[:], wq_d[:, :, :, :])
            wk_sb = apool.tile([128, 2, CCH, 128], BF16)
            nc.gpsimd.dma_start(wk_sb[:], wk_d[:, :, :, :])
            wv_sb = apool.tile([128, CCH, 256], BF16)
            nc.gpsimd.dma_start(wv_sb[:], wv_d[:, :, :])
            wq9_sb = apool.tile([2, 2, 128], BF16)
            nc.gpsimd.dma_start(wq9_sb[:], wq9_d[:, :, :])
            wk9_sb = apool.tile([2, 2, 128], BF16)
            nc.gpsimd.dma_start(wk9_sb[:], wk9_d[:, :, :])
            wv9_sb = apool.tile([2, 256], BF16)
            nc.gpsimd.dma_start(wv9_sb[:], wv9_d[:, :])
            wp_sb = apool.tile([128, 2, 1024], BF16)
            nc.gpsimd.dma_start(wp_sb[:], wp_d[:, :, :])
            tri = apool.tile([128, 512], BF16)
            nc.gpsimd.dma_start(tri[:], mk_d[:, :])

            qT = apool.tile([128, HL, T], BF16)
            kT = apool.tile([128, HL, T], BF16)
            nc.gpsimd.memset(qT[:], 0.0)
            nc.gpsimd.memset(kT[:], 0.0)
            Vh = apool.tile([128, NTT, HL, 66], BF16)
            nc.gpsimd.memset(Vh[:, :, :, 65:66], 0.0)
            nc.gpsimd.dma_start(Vh[:, :, :, 64:65], ckb_d[:, :, :])

            dpad2s = []
            for i in range(2):
                dp = apool.tile([128, 512], F32, name=f"dpad{i}")
                nc.gpsimd.memset(dp[:], 0.0)
                dpad2s.append(dp)

            with ExitStack() as qkv_scope:
                qpool = qkv_scope.enter_context(tc.tile_pool(name="qkv", bufs=1))
                psQ = qkv_scope.enter_context(
                    tc.tile_pool(name="psQ", bufs=4, space="PSUM"))
                psB = qkv_scope.enter_context(
                    tc.tile_pool(name="psB", bufs=1, space="PSUM"))
                psV = qkv_scope.enter_context(
                    tc.tile_pool(name="psV", bufs=2, space="PSUM"))

                xT_sb = qpool.tile([128, CCH, T], BF16)
                for cc in range(CCH):
                    nc.sync.dma_start(xT_sb[:, cc, :], xT_d[:, cc, :])

                mrow = qpool.tile([2, T], BF16)    # row0: -mu, row1: ones(b1)
                nc.vector.memset(mrow[:], 0.0)
                if has_b1:
                    nc.vector.memset(mrow[1:2, :], 1.0)
                rstd_row = qpool.tile([1, T], F32)
                rstdb = [qpool.tile([128, 512], F32, name=f"rstdb{b}")
                         for b in range(NTB)]
                rstds = [qpool.tile([128, 1], F32, name=f"rstds{t}")
                         for t in range(NTT)]

                # ---- per-token-tile LN stats (vector) --------------------
                mucB = [qpool.tile([128, 32], F32, name=f"mucB{b}")
                        for b in range(NTB)]
                for b in range(NTB):
                    nc.gpsimd.memset(mucB[b][:], 0.0)
                for t in range(NTT):
                    xt = work.tile([128, C], BF16, tag="xt")
                    nc.scalar.dma_start(xt[:], xtok_d[ts(t, 128), :])
                    bst = stats.tile([128, 2, 6], F32, tag="bst")
                    for sg in range(2):
                        nc.vector.bn_stats(bst[:, sg, :], xt[:, ts(sg, 512)])
                    mv = stats.tile([128, 2], F32, tag="mv")
                    nc.vector.bn_aggr(mv[:], bst[:])
                    std = stats.tile([128, 1], F32, tag="std")
                    nc.scalar.activation(std[:], mv[:, 1:2],
                                         mybir.ActivationFunctionType.Sqrt,
                                         bias=eps_ap[:])
                    nc.vector.reciprocal(rstds[t][:], std[:])
                    b, i = t // 4, t % 4
                    nc.vector.tensor_scalar_mul(mucB[b][:, i:i + 1],
                                                mv[:, 0:1], -1.0)
                    nc.vector.tensor_copy(mucB[b][:, 4 + i:5 + i], rstds[t][:])

                for b in range(NTB):
                    stt = stats.tile([128, 32], F32, tag="stt")
                    nc.vector.transpose(stt[:], mucB[b][:])
                    sTb = stats.tile([8, 128], BF16, tag="sTb")
                    sTf = stats.tile([8, 128], F32, tag="sTf")
                    for i in range(4):
                        nc.vector.tensor_copy(sTb[0:8, ts(i, 32)],
                                              stt[32 * i:32 * i + 8, :])
                        nc.vector.tensor_copy(sTf[0:8, ts(i, 32)],
                                              stt[32 * i:32 * i + 8, :])
                    for i in range(4):
                        nc.gpsimd.dma_start(
                            mrow[0:1, 512 * b + 128 * i:512 * b + 128 * (i + 1)],
                            sTb[i:i + 1, :])
                        nc.gpsimd.dma_start(
                            rstd_row[0:1, 512 * b + 128 * i:512 * b + 128 * (i + 1)],
                            sTf[4 + i:5 + i, :])

                # ---- QKV -------------------------------------------------
                for b in range(NTB):
                    pss = {}
                    for p in range(2):
                        for wi in range(2):
                            pss[(p, wi)] = psQ.tile(
                                [128, 512], F32, tag="qk",
                                name=f"qk_{b}_{p}_{wi}")
                    for cc in range(CCH):
                        for p in range(2):
                            for wi, wsb in enumerate((wq_sb, wk_sb)):
                                nc.tensor.matmul(
                                    pss[(p, wi)][:],
                                    wsb[:, p, cc, :],
                                    xT_sb[:, cc, ts(b, 512)],
                                    start=(cc == 0), stop=False)
                    for p in range(2):
                        for wi, w9 in enumerate((wq9_sb, wk9_sb)):
                            nc.tensor.matmul(
                                pss[(p, wi)][:], w9[0:K9, p, :],
                                mrow[0:K9, ts(b, 512)],
                                start=False, stop=True)
                    psb = psB.tile([128, 512], F32, tag="psb")
                    nc.tensor.matmul(psb[:], ones1[0:1, :],
                                     rstd_row[0:1, ts(b, 512)],
                                     start=True, stop=True)
                    nc.scalar.copy(rstdb[b][:], psb[:])
                    for p in range(2):
                        for wi, dstT in enumerate((qT, kT)):
                            for j in range(2):
                                nc.vector.tensor_tensor(
                                    dstT[0:64, 2 * p + j, ts(b, 512)],
                                    pss[(p, wi)][64 * j:64 * j + 64, :],
                                    rstdb[b][64 * j:64 * j + 64, :],
                                    mybir.AluOpType.mult)

                for t in range(NTT):
                    psv = psV.tile([128, 256], F32, tag="v")
                    for cc in range(CCH):
                        nc.tensor.matmul(psv[:], xT_sb[:, cc, ts(t, 128)],
                                         wv_sb[:, cc, :],
                                         start=(cc == 0), stop=False)
                    nc.tensor.matmul(psv[:], mrow[0:K9, ts(t, 128)],
                                     wv9_sb[0:K9, :], start=False, stop=True)
                    for h in range(HL):
                        nc.vector.tensor_scalar(
                            Vh[:, t, h, 0:64], psv[:, ts(h, 64)],
                            rstds[t][:], ck32[:, t, h:h + 1],
                            mybir.AluOpType.mult, mybir.AluOpType.mult)

            # ---- attention + per-block proj/RS + deferred LN2 -----------
            with ExitStack() as ps_scope:
                psS = ps_scope.enter_context(
                    tc.tile_pool(name="psS", bufs=3, space="PSUM"))
                psO = ps_scope.enter_context(
                    tc.tile_pool(name="psO", bufs=3, space="PSUM"))

                def emit_proj(c):
                    # proj partial over local heads -> DRAM -> RS
                    for i2 in range(4):
                        for nb in range(2):
                            pp = psS.tile([128, 512], F32, tag="pp", bufs=2,
                                          name=f"pp_{c}_{nb}_{i2}")
                            for cc in range(2):
                                nc.tensor.matmul(pp[:],
                                                 OnT_cs[c][:, cc, ts(i2, 128)],
                                                 wp_sb[:, cc, ts(nb, 512)],
                                                 start=(cc == 0), stop=(cc == 1))
                            pjs = pipe.tile([128, 512], BF16, tag="pjs",
                                            bufs=2, name=f"pjs_{c}_{nb}_{i2}")
                            nc.scalar.copy(pjs[:], pp[:])
                            nc.sync.dma_start(
                                rs_in[c][ts(i2, 128), ts(nb, 512)], pjs[:])
                    nc.gpsimd.collective_compute(
                        "ReduceScatter", mybir.AluOpType.add,
                        replica_groups=GROUPS,
                        ins=[rs_in[c].opt()], outs=[rs_out[c].opt()],
                    )

                def emit_ln2(c):
                    xrt = work.tile([128, C], F32, tag="xrt", bufs=2)
                    nc.sync.dma_start(xrt[:], xr_d[ts(c, 128), :])
                    rst = work.tile([128, C], BF16, tag="rst", bufs=2)
                    nc.sync.dma_start(rst[:], rs_out[c][:, :])
                    nc.vector.tensor_tensor(x2[:, c, :], rst[:], xrt[:],
                                            mybir.AluOpType.add)
                    bst = stats.tile([128, 2, 6], F32, tag="bst2")
                    for sg in range(2):
                        nc.vector.bn_stats(bst[:, sg, :], x2[:, c, ts(sg, 512)])
                    nc.vector.bn_aggr(mv2[c][:], bst[:])
                    std = stats.tile([128, 1], F32, tag="std2")
                    nc.scalar.activation(std[:], mv2[c][:, 1:2],
                                         mybir.ActivationFunctionType.Sqrt,
                                         bias=eps_ap[:])
                    rstd2 = stats.tile([128, 1], F32, tag="rstd2")
                    nc.vector.reciprocal(rstd2[:], std[:])
                    yb = work.tile([128, C], BF16, tag="yb", bufs=2)
                    nc.vector.tensor_scalar(yb[:], x2[:, c, :], mv2[c][:, 0:1],
                                            rstd2[:], mybir.AluOpType.subtract,
                                            mybir.AluOpType.mult)
                    for half in range(2):
                        pt = psS.tile([128, 4, 128], BF16, tag="pp", bufs=2,
                                      name=f"tr2_{c}_{half}")
                        for i in range(4):
                            cc = half * 4 + i
                            nc.tensor.transpose(pt[:, i, :], yb[:, ts(cc, 128)],
                                                ident[:])
                        nc.vector.tensor_copy(
                            y2T8[:, half * 4:(half + 1) * 4, ts(c, 128)], pt[:])

                npair = [0]
                prev_c = None
                OnT_cs = {}
                for c in (3, 2, 1, 0):
                    OnT_c = OnTp.tile([128, 2, 512], BF16, tag="OnT")
                    OnT_cs[c] = OnT_c
                    t_hi = 4 * c + 4
                    for p in range(2):
                        pos = [psO.tile([66, 512], F32, tag="po",
                                        name=f"po{j}_{c}_{p}") for j in range(2)]
                        for j in range(2):
                            h = 2 * p + j
                            po = pos[j]

                            def emit_s(t):
                                if t >= 4 * c:
                                    w = 512 - 128 * (t - 4 * c)
                                    diag = True
                                else:
                                    w, diag = 512, False
                                q0 = 512 * c + 512 - w
                                st = psS.tile([128, 512], F32, tag="s",
                                              name=f"s_{c}_{h}_{t}")
                                nc.tensor.matmul(st[:, 0:w],
                                                 kT[:, h, ts(t, 128)],
                                                 qT[:, h, q0:q0 + w],
                                                 start=True, stop=True)
                                pT = pipe.tile([128, 512], BF16, tag="pT",
                                               name=f"pT_{c}_{h}_{t}")
                                nc.scalar.activation(
                                    pT[:, 0:w], st[:, 0:w],
                                    mybir.ActivationFunctionType.Exp,
                                    bias=cb24[:])
                                if diag:
                                    nc.vector.tensor_tensor(
                                        pT[:, 0:w], pT[:, 0:w],
                                        tri[:, 0:w], mybir.AluOpType.mult)
                                return pT, w

                            pTs = {0: emit_s(0)}
                            if t_hi > 1:
                                pTs[1] = emit_s(1)
                            for t in range(t_hi):
                                pT, w = pTs.pop(t)
                                nc.tensor.matmul(po[:, 512 - w:512],
                                                 Vh[:, t, h, 0:66],
                                                 pT[:, 0:w], start=(t == 0),
                                                 stop=(t == t_hi - 1),
                                                 skip_group_check=True)
                                if t + 2 < t_hi:
                                    pTs[t + 2] = emit_s(t + 2)
                        # softmax denominators for the pair -> broadcast
                        dp = dpad2s[npair[0] % 2]
                        npair[0] += 1
                        nc.vector.tensor_copy(dp[0:1, :], pos[0][64:65, :])
                        nc.vector.tensor_copy(dp[64:65, :], pos[1][64:65, :])
                        rb = psS.tile([128, 512], F32, tag="s",
                                      name=f"rb_{c}_{p}")
                        nc.tensor.matmul(rb[:], sel2[:], dp[:],
                                         start=True, stop=True)
                        rec = recp.tile([128, 512], F32, tag="rec")
                        nc.vector.reciprocal(rec[:], rb[:])
                        nc.vector.tensor_tensor(OnT_c[0:64, p, :],
                                                pos[0][0:64, :], rec[0:64, :],
                                                mybir.AluOpType.mult)
                        nc.vector.tensor_tensor(OnT_c[64:128, p, :],
                                                pos[1][0:64, :],
                                                rec[64:128, :],
                                                mybir.AluOpType.mult)
                    emit_proj(c)
                    if prev_c is not None:
                        emit_ln2(prev_c)
                    prev_c = c
                emit_ln2(0)

        # ---- SwiGLU MLP (row-parallel, fp8 DoubleRow) -------------------
        with ExitStack() as mlp_scope:
            mpool = mlp_scope.enter_context(tc.tile_pool(name="mlp", bufs=1))
            wstream = mlp_scope.enter_context(tc.tile_pool(name="wstream", bufs=3))
            psC = mlp_scope.enter_context(
                tc.tile_pool(name="psC", bufs=2, space="PSUM"))

            gu = mpool.tile([128, MCH, ROWS], F8)
            db_sb = mpool.tile([128, 1024], F32)
            nc.gpsimd.dma_start(db_sb[:], db_d[:, :])
            wd_sb = mpool.tile([128, MCH, 1024], F8)
            for half in range(2):
                nc.gpsimd.dma_start(wd_sb[:, :, ts(half, 512)],
                                    wd_d[:, :, ts(half, 512)])

            for hc in range(MCH):
                wgt = wstream.tile([128, CCH, 128], F8, tag="wgt")
                nc.sync.dma_start(wgt[:], wg_d[:, hc, :, :])
                wut = wstream.tile([128, CCH, 128], F8, tag="wut")
                nc.sync.dma_start(wut[:], wu_d[:, hc, :, :])
                pg = psC.tile([128, 512], F32, tag="g")
                pu = psC.tile([128, 512], F32, tag="u")
                for c2 in range(CCH // 2):
                    nc.tensor.matmul(pg[:], wgt[:, 2 * c2:2 * c2 + 2, :],
                                     y2T8[:, 2 * c2:2 * c2 + 2, :],
                                     start=(c2 == 0), stop=(c2 == CCH // 2 - 1),
                                     perf_mode=DR)
                for c2 in range(CCH // 2):
                    nc.tensor.matmul(pu[:], wut[:, 2 * c2:2 * c2 + 2, :],
                                     y2T8[:, 2 * c2:2 * c2 + 2, :],
                                     start=(c2 == 0), stop=(c2 == CCH // 2 - 1),
                                     perf_mode=DR)
                gs = work.tile([128, 512], BF16, tag="gs")
                nc.scalar.activation(gs[:], pg[:],
                                     mybir.ActivationFunctionType.Silu,
                                     bias=bg_sb[:, hc:hc + 1])
                us = work.tile([128, 512], BF16, tag="us")
                nc.scalar.activation(us[:], pu[:],
                                     mybir.ActivationFunctionType.Identity,
                                     bias=bu_sb[:, hc:hc + 1])
                nc.vector.tensor_tensor(gu[:, hc, :], gs[:], us[:],
                                        mybir.AluOpType.mult)

            for nb in range(2):
                for tt in range(RT):
                    pd = psC.tile([128, 512], F32, tag="d")
                    for h2 in range(MCH // 2):
                        nc.tensor.matmul(pd[:],
                                         gu[:, 2 * h2:2 * h2 + 2, ts(tt, 128)],
                                         wd_sb[:, 2 * h2:2 * h2 + 2, ts(nb, 512)],
                                         start=(h2 == 0),
                                         stop=(h2 == MCH // 2 - 1),
                                         perf_mode=DR)
                    o1 = work.tile([128, 512], F32, tag="o1")
                    nc.vector.tensor_tensor(o1[:], pd[:],
                                            x2[:, tt, ts(nb, 512)],
                                            mybir.AluOpType.add)
                    nc.vector.tensor_tensor(o1[:], o1[:],
                                            db_sb[:, ts(nb, 512)],
                                            mybir.AluOpType.add)
                    nc.sync.dma_start(out_d[ts(tt, 128), ts(nb, 512)], o1[:])

    _split_excess_waits(nc)
    return nc


# ----------------------------------------------------------------------------
# host-side input prep + launch
# ----------------------------------------------------------------------------
_cache = {}

def _get_nc(has_b1):
    if has_b1 not in _cache:
        _cache[has_b1] = _build(has_b1)
    return _cache[has_b1]


def _prep(x, ln1_g, ln1_b, qkv_w, qkv_b, proj_w, proj_b,
          ln2_g, ln2_b, gate_w, gate_b, up_w, up_b, down_w, down_b):
    x = np.asarray(x, np.float32)
    f = lambda a: np.asarray(a, np.float32)
    ln1_g, ln1_b, qkv_b, proj_b, ln2_g, ln2_b = map(f, (
        ln1_g, ln1_b, qkv_b, proj_b, ln2_g, ln2_b))
    qkv_w, proj_w, gate_w, gate_b, up_w, up_b, down_w, down_b = map(f, (
        qkv_w, proj_w, gate_w, gate_b, up_w, up_b, down_w, down_b))

    slopes = _slopes()

    # fold LN affines into the consuming matmuls
    w1 = qkv_w * ln1_g[:, None]
    b1 = ln1_b @ qkv_w + qkv_b              # [3C]
    wg_f = gate_w * ln2_g[:, None]
    bg_f = ln2_b @ gate_w + gate_b          # [HID]
    wu_f = up_w * ln2_g[:, None]
    bu_f = ln2_b @ up_w + up_b

    has_b1 = bool(np.any(b1 != 0.0))

    wgp = np.zeros((C, HIDP), np.float32); wgp[:, :HID] = wg_f
    wup = np.zeros((C, HIDP), np.float32); wup[:, :HID] = wu_f
    wdp = np.zeros((HIDP, 1024), np.float32); wdp[:HID] = down_w
    bgp = np.zeros(HIDP, np.float32); bgp[:HID] = bg_f
    bup = np.zeros(HIDP, np.float32); bup[:HID] = bu_f

    wg_dev = wgp.reshape(CCH, 128, MCH, 128).transpose(1, 2, 0, 3).astype(f8np)
    wu_dev = wup.reshape(CCH, 128, MCH, 128).transpose(1, 2, 0, 3).astype(f8np)
    wd_dev = wdp.reshape(MCH, 128, 1024).transpose(1, 0, 2).astype(f8np)
    bg_dev = bgp.reshape(MCH, 128).T.copy()
    bu_dev = bup.reshape(MCH, 128).T.copy()
    db_dev = np.broadcast_to(down_b, (128, 1024)).copy()

    # triangular diagonal mask (query >= key within a trimmed diag tile)
    pp_i = np.arange(128)[:, None]
    jj = np.arange(512)[None, :]
    tri_np = (jj >= pp_i).astype(bf16)       # [128, 512]

    sel2_np = np.zeros((128, 128), np.float32)
    sel2_np[0, 0:64] = 1.0
    sel2_np[64, 64:128] = 1.0

    def qkv_w9(wcols, bcols):
        w9 = np.zeros((2, 2, 128), np.float32)
        w9[0] = wcols.sum(axis=0).reshape(2, 128)
        w9[1] = bcols.reshape(2, 128)
        return w9.astype(bf16)

    in_maps = []
    for core in range(N_CORES):
        b, g = core // TP, core % TP
        heads = range(4 * g, 4 * g + 4)
        qcols = np.concatenate([np.arange(h * D, (h + 1) * D) for h in heads])
        kcols = qcols + C
        vcols = qcols + 2 * C

        wq_cols = w1[:, qcols] * 0.125           # [C, 256]
        wk_cols = w1[:, kcols]
        wv_cols = w1[:, vcols]
        wq_dev = wq_cols.reshape(CCH, 128, 2, 128).transpose(1, 2, 0, 3).astype(bf16)
        wk_dev = wk_cols.reshape(CCH, 128, 2, 128).transpose(1, 2, 0, 3).astype(bf16)
        wv_dev = wv_cols.reshape(CCH, 128, 256).transpose(1, 0, 2).astype(bf16)
        wq9_dev = qkv_w9(wq_cols, b1[qcols] * 0.125)
        wk9_dev = qkv_w9(wk_cols, b1[kcols])
        wv9_dev = np.zeros((2, 256), np.float32)
        wv9_dev[0] = wv_cols.sum(axis=0)
        wv9_dev[1] = b1[vcols]
        wv9_dev = wv9_dev.astype(bf16)

        wp_rows = proj_w[qcols, :]                        # [256, 1024]
        wp_dev = wp_rows.reshape(2, 128, 1024).transpose(1, 0, 2).astype(bf16)

        # ALiBi key-side factors folded into V (and the denominator column)
        ck = np.zeros((128, NTT, HL), np.float64)
        for hl, h in enumerate(heads):
            sl = slopes[h]
            for t in range(NTT):
                ck[:, t, hl] = np.exp(-sl * (128 * t + np.arange(128)))
        ckf = ck.astype(np.float32)

        xb = x[b]                                # [T, C]
        xT_dev = np.ascontiguousarray(
            xb.T.reshape(CCH, 128, T).transpose(1, 0, 2)).astype(bf16)

        in_maps.append({
            "xT": xT_dev,
            "xtok": xb.astype(bf16),
            "x_rows": np.concatenate(
                [xb[512 * r + 128 * g:512 * r + 128 * g + 128]
                 for r in range(4)], axis=0) + proj_b[None, :],
            "wq": wq_dev, "wk": wk_dev, "wv": wv_dev,
            "wq9": wq9_dev, "wk9": wk9_dev, "wv9": wv9_dev,
            "wproj": wp_dev,
            "wg": wg_dev, "wu": wu_dev, "wd": wd_dev,
            "bg": bg_dev, "bu": bu_dev, "dbias": db_dev,
            "masks": tri_np, "ckb": ckf.astype(bf16), "ckf": ckf,
            "sel2": sel2_np,
        })

    return has_b1, in_maps


def _gather(results):
    out = np.empty((B, T, C), np.float32)
    for core in range(N_CORES):
        b, g = core // TP, core % TP
        orr = results[core]["out_rows"]
        for r in range(4):
            out[b, 512 * r + 128 * g:512 * r + 128 * g + 128] = \
                orr[128 * r:128 * (r + 1)]
    return out


def kernel(**inputs):
    has_b1, in_maps = _prep(**inputs)
    nc = _get_nc(has_b1)
    res = bass_utils.run_bass_kernel_spmd(
        nc, in_maps, core_ids=list(range(N_CORES)))
    return _gather(res.results)


# revision 25
# speedup vs baseline: 1.1943x; 1.0617x over previous
"""Trainium2 Bass kernel for EnhancedTransformerBlock (B=2,T=2048,C=1024,H=16,
SwiGLU HIDDEN=2730, ALiBi-abs + causal attention).

Sharding over 8 cores: batch (2) x head-groups (4 heads/core). Host ships x
both token-major (bf16, LN stats only) and feature-major (bf16, matmul
stream); LN1 is folded into the QKV matmuls via a mean-row augmentation plus
a per-token rstd column scale applied on the PSUM->SBUF copy. Attention runs
with transposed scores S^T[tk,tq]; the ALiBi key-side bias is folded into
V's rows (so the exp bias is a shared constant), the softmax denominator
rides as a scaled ones column through the PV matmul, and causally-dead query
columns are trimmed from the diagonal S/exp/PV tiles. proj partials are
ReduceScattered per 512-token block (bf16); the SwiGLU MLP runs row-parallel
with fp8(e4m3) DoubleRow matmuls. Host reassembles row blocks.
"""
import sys, types
sys.path.insert(0, "/opt/trn_rl_repo")
import numpy as np
import ml_dtypes

import concourse.bass as bass
import concourse.tile as tile
from concourse import mybir
import concourse.bass_utils as bass_utils
import bass_rust

# ----------------------------------------------------------------------------
# environment patches (walrus in this container accepts only 1 sync-wait/inst)
# ----------------------------------------------------------------------------
_DRAIN_WAIT_LIMIT = 1

def _patched_drain_and_barrier(self, tick_clock, wait_clock):
    nc = self.nc
    drain_inst = nc.sync.drain()
    wait_clock.add_sem_waits(
        drain_inst.ins, bass_rust.ScopedClock({None: tick_clock.global_clock})
    )
    si = drain_inst.ins.sync_info
    waits = list(si.on_wait) if si is not None else []
    if len(waits) > _DRAIN_WAIT_LIMIT:
        si.on_wait = waits[:_DRAIN_WAIT_LIMIT]
        for i in range(_DRAIN_WAIT_LIMIT, len(waits), _DRAIN_WAIT_LIMIT):
            d2 = nc.sync.drain()
            d2.ins.sync_info = bass_rust.SyncInfo(
                on_wait=waits[i:i + _DRAIN_WAIT_LIMIT], on_update=[]
            )
    nc.all_engine_barrier()
    popped = nc._tile_sem_poison_stack.pop()
    assert popped is self._sem_poison
    nc.clear_and_free_semaphores(list(self.sems.allocated().values()))
    nc.all_engine_barrier()


def _split_excess_waits(nc, limit=_DRAIN_WAIT_LIMIT):
    n = [0]
    for bb in nc.main_func.blocks:
        insts = bb.instructions
        out = []
        changed = False
        for inst in insts:
            si = inst.sync_info
            waits = list(si.on_wait) if si is not None else []
            if len(waits) > limit:
                changed = True
                keep = waits[-limit:]
                rest = waits[:-limit]
                for i in range(0, len(rest), limit):
                    n[0] += 1
                    d = mybir.InstNoOp(
                        name=f"waitsplit-{n[0]}", engine=inst.engine, ins=[], outs=[]
                    )
                    d.sync_info = bass_rust.SyncInfo(
                        on_wait=rest[i:i + limit], on_update=[]
                    )
                    out.append(d)
                si.on_wait = keep
            out.append(inst)
        if changed:
            bb.instructions = out


def _install_patches():
    tile.TileContext._drain_and_barrier = _patched_drain_and_barrier
    if "antenv.axon_hooks" not in sys.modules:
        try:
            from trn_agent_boot.trn_boot import _ntff_profile_via_ctypes
            hook = _ntff_profile_via_ctypes("/opt/axon/libaxon_pjrt.so")
        except Exception:
            hook = None
        mod = types.ModuleType("antenv.axon_hooks")
        mod.get_axon_ntff_profile_hook = lambda: hook
        mod.set_axon_ntff_profile_hook = lambda h: None
        sys.modules["antenv.axon_hooks"] = mod
        bass_utils.upload_artifacts = lambda tmpdir: tmpdir

_install_patches()

# ----------------------------------------------------------------------------
# problem constants
# ----------------------------------------------------------------------------
B, T, C = 2, 2048, 1024
H, D = 16, 64
HID, HIDP = 2730, 2816
N_CORES, TP = 8, 4
ROWS = T // TP          # 512 rows per core after the reduce-scatter
HL = 4                  # local heads per core
EPS = 1e-5
CBIAS = 24.0
F32, BF16 = mybir.dt.float32, mybir.dt.bfloat16
F8 = mybir.dt.float8e4
bf16 = ml_dtypes.bfloat16
f8np = ml_dtypes.float8_e4m3
ts = bass.ts

NTT = T // 128          # 16 token tiles
NTB = T // 512          # 4 token blocks
CCH = C // 128          # 8 feature chunks
MCH = HIDP // 128       # 22 hidden chunks
RT = ROWS // 128        # 4 row tiles per core
GROUPS = [[0, 1, 2, 3], [4, 5, 6, 7]]
DR = mybir.MatmulPerfMode.DoubleRow


def _slopes():
    i = np.arange(1, H + 1, dtype=np.float64)
    return 1.0 / np.power(2.0, 8.0 * i / H)


# ----------------------------------------------------------------------------
# device program (identical on all 8 cores; per-core behavior via input data)
# ----------------------------------------------------------------------------
def _build(has_b1):
    K9 = 2 if has_b1 else 1
    nc = bass.Bass("TRN2", num_devices=N_CORES)

    xT_d = nc.dram_tensor("xT", [128, CCH, T], BF16, kind="ExternalInput")
    xtok_d = nc.dram_tensor("xtok", [T, C], BF16, kind="ExternalInput")
    xr_d = nc.dram_tensor("x_rows", [ROWS, C], F32, kind="ExternalInput")
    wq_d = nc.dram_tensor("wq", [128, 2, CCH, 128], BF16, kind="ExternalInput")
    wk_d = nc.dram_tensor("wk", [128, 2, CCH, 128], BF16, kind="ExternalInput")
    wv_d = nc.dram_tensor("wv", [128, CCH, 256], BF16, kind="ExternalInput")
    wq9_d = nc.dram_tensor("wq9", [2, 2, 128], BF16, kind="ExternalInput")
    wk9_d = nc.dram_tensor("wk9", [2, 2, 128], BF16, kind="ExternalInput")
    wv9_d = nc.dram_tensor("wv9", [2, 256], BF16, kind="ExternalInput")
    wp_d = nc.dram_tensor("wproj", [128, 2, 1024], BF16, kind="ExternalInput")
    wg_d = nc.dram_tensor("wg", [128, MCH, CCH, 128], F8, kind="ExternalInput")
    wu_d = nc.dram_tensor("wu", [128, MCH, CCH, 128], F8, kind="ExternalInput")
    wd_d = nc.dram_tensor("wd", [128, MCH, 1024], F8, kind="ExternalInput")
    bg_d = nc.dram_tensor("bg", [128, MCH], F32, kind="ExternalInput")
    bu_d = nc.dram_tensor("bu", [128, MCH], F32, kind="ExternalInput")
    db_d = nc.dram_tensor("dbias", [128, 1024], F32, kind="ExternalInput")
    mk_d = nc.dram_tensor("masks", [128, 512], BF16, kind="ExternalInput")
    ckb_d = nc.dram_tensor("ckb", [128, NTT, HL], BF16, kind="ExternalInput")
    ckf_d = nc.dram_tensor("ckf", [128, NTT, HL], F32, kind="ExternalInput")
    sel2_d = nc.dram_tensor("sel2", [128, 128], BF16, kind="ExternalInput")

    out_d = nc.dram_tensor("out_rows", [ROWS, C], F32, kind="ExternalOutput")

    from contextlib import ExitStack
    with tile.TileContext(nc) as tc, ExitStack() as top:
        consts = top.enter_context(tc.tile_pool(name="consts", bufs=1))
        stats = top.enter_context(tc.tile_pool(name="stats", bufs=16))
        work = top.enter_context(tc.tile_pool(name="work", bufs=3))
        dramp = top.enter_context(tc.tile_pool(name="dram", bufs=1, space="DRAM"))

        eps_ap = consts.tile([128, 1], F32)
        nc.vector.memset(eps_ap[:], EPS)
        cb24 = consts.tile([128, 1], F32)
        nc.vector.memset(cb24[:], -CBIAS)
        ones1 = consts.tile([1, 128], F32)
        nc.vector.memset(ones1[:], 1.0)
        sel2 = consts.tile([128, 128], BF16)
        nc.gpsimd.dma_start(sel2[:], sel2_d[:, :])
        ck32 = consts.tile([128, NTT, HL], F32)
        nc.gpsimd.dma_start(ck32[:], ckf_d[:, :, :])
        bg_sb = consts.tile([128, MCH], F32)
        nc.gpsimd.dma_start(bg_sb[:], bg_d[:, :])
        bu_sb = consts.tile([128, MCH], F32)
        nc.gpsimd.dma_start(bu_sb[:], bu_d[:, :])
        ident = consts.tile([128, 128], BF16)
        from concourse.masks import make_identity
        make_identity(nc, ident[:])

        # persistent cross-phase state
        x2 = consts.tile([128, RT, C], F32)
        y2T8 = consts.tile([128, CCH, ROWS], F8)
        mv2 = [consts.tile([128, 2], F32, name=f"mv2_{r}") for r in range(RT)]

        rs_in = [dramp.tile([512, C], BF16, name=f"rsin{i}") for i in range(NTB)]
        rs_out = [dramp.tile([128, C], BF16, name=f"rsout{i}")
                  for i in range(NTB)]

        with ExitStack() as attn_scope:
            apool = attn_scope.enter_context(tc.tile_pool(name="attn", bufs=1))
            pipe = attn_scope.enter_context(tc.tile_pool(name="pipe", bufs=4))
            OnTp = attn_scope.enter_context(tc.tile_pool(name="OnTp", bufs=2))
            recp = attn_scope.enter_context(tc.tile_pool(name="recp", bufs=2))

            qT = apool.tile([128, HL, T], BF16)
            kT = apool.tile([128, HL, T], BF16)
            nc.gpsimd.memset(qT[64:128, :, :], 0.0)
            nc.gpsimd.memset(kT[64:128, :, :], 0.0)
            Vh = apool.tile([128, NTT, HL, 66], BF16)
            nc.gpsimd.memset(Vh[:, :, :, 65:66], 0.0)
            nc.gpsimd.dma_start(Vh[:, :, :, 64:65], ckb_d[:, :, :])
            dpad2s = []
            for i in range(2):
                dp = apool.tile([128, 512], BF16, name=f"dpad{i}")
                nc.vector.memset(dp[:], 0.0)
                dpad2s.append(dp)

            wq_sb = apool.tile([128, 2, CCH, 128], BF16)
            nc.gpsimd.dma_start(wq_sb[:], wq_d[:, :, :, :])
            wk_sb = apool.tile([128, 2, CCH, 128], BF16)
            nc.gpsimd.dma_start(wk_sb[:], wk_d[:, :, :, :])
            wv_sb = apool.tile([128, CCH, 256], BF16)
            nc.gpsimd.dma_start(wv_sb[:], wv_d[:, :, :])
            wq9_sb = apool.tile([2, 2, 128], BF16)
            nc.gpsimd.dma_start(wq9_sb[:], wq9_d[:, :, :])
            wk9_sb = apool.tile([2, 2, 128], BF16)
            nc.gpsimd.dma_start(wk9_sb[:], wk9_d[:, :, :])
            wv9_sb = apool.tile([2, 256], BF16)
            nc.gpsimd.dma_start(wv9_sb[:], wv9_d[:, :])
            wp_sb = apool.tile([128, 2, 1024], BF16)
            nc.gpsimd.dma_start(wp_sb[:], wp_d[:, :, :])
            tri = apool.tile([128, 512], BF16)
            nc.gpsimd.dma_start(tri[:], mk_d[:, :])

            with ExitStack() as qkv_scope:
                qpool = qkv_scope.enter_context(tc.tile_pool(name="qkv", bufs=1))
                psQ = qkv_scope.enter_context(
                    tc.tile_pool(name="psQ", bufs=4, space="PSUM"))
                psB = qkv_scope.enter_context(
                    tc.tile_pool(name="psB", bufs=1, space="PSUM"))
                psV = qkv_scope.enter_context(
                    tc.tile_pool(name="psV", bufs=2, space="PSUM"))

                xT_sb = qpool.tile([128, CCH, T], BF16)
                for cc in range(CCH):
                    nc.sync.dma_start(xT_sb[:, cc, :], xT_d[:, cc, :])

                mrow = qpool.tile([2, T], BF16)    # row0: -mu, row1: ones(b1)
                nc.vector.memset(mrow[:], 0.0)
                if has_b1:
                    nc.vector.memset(mrow[1:2, :], 1.0)
                rstd_row = qpool.tile([1, T], F32)
                rstdb = [qpool.tile([128, 512], F32, name=f"rstdb{b}")
                         for b in range(NTB)]
                rstds = [qpool.tile([128, 1], F32, name=f"rstds{t}")
                         for t in range(NTT)]

                # ---- per-token-tile LN stats (vector) --------------------
                mucB = [qpool.tile([128, 32], F32, name=f"mucB{b}")
                        for b in range(NTB)]
                for b in range(NTB):
                    nc.vector.memset(mucB[b][:], 0.0)
                for t in range(NTT):
                    xt = work.tile([128, C], BF16, tag="xt")
                    nc.scalar.dma_start(xt[:], xtok_d[ts(t, 128), :])
                    bst = stats.tile([128, 2, 6], F32, tag="bst")
                    for sg in range(2):
                        nc.vector.bn_stats(bst[:, sg, :], xt[:, ts(sg, 512)])
                    mv = stats.tile([128, 2], F32, tag="mv")
                    nc.vector.bn_aggr(mv[:], bst[:])
                    std = stats.tile([128, 1], F32, tag="std")
                    nc.scalar.activation(std[:], mv[:, 1:2],
                                         mybir.ActivationFunctionType.Sqrt,
                                         bias=eps_ap[:])
                    nc.vector.reciprocal(rstds[t][:], std[:])
                    b, i = t // 4, t % 4
                    nc.vector.tensor_scalar_mul(mucB[b][:, i:i + 1],
                                                mv[:, 0:1], -1.0)
                    nc.vector.tensor_copy(mucB[b][:, 4 + i:5 + i], rstds[t][:])

                for b in range(NTB):
                    stt = stats.tile([128, 32], F32, tag="stt")
                    nc.vector.transpose(stt[:], mucB[b][:])
                    sTb = stats.tile([8, 128], BF16, tag="sTb")
                    sTf = stats.tile([8, 128], F32, tag="sTf")
                    for i in range(4):
                        nc.vector.tensor_copy(sTb[0:8, ts(i, 32)],
                                              stt[32 * i:32 * i + 8, :])
                        nc.vector.tensor_copy(sTf[0:8, ts(i, 32)],
                                              stt[32 * i:32 * i + 8, :])
                    for i in range(4):
                        nc.sync.dma_start(
                            mrow[0:1, 512 * b + 128 * i:512 * b + 128 * (i + 1)],
                            sTb[i:i + 1, :])
                        nc.sync.dma_start(
                            rstd_row[0:1, 512 * b + 128 * i:512 * b + 128 * (i + 1)],
                            sTf[4 + i:5 + i, :])

                # ---- QKV -------------------------------------------------
                for b in range(NTB):
                    pss = {}
                    for p in range(2):
                        for wi in range(2):
                            pss[(p, wi)] = psQ.tile(
                                [128, 512], F32, tag="qk",
                                name=f"qk_{b}_{p}_{wi}")
                    for cc in range(CCH):
                        for p in range(2):
                            for wi, wsb in enumerate((wq_sb, wk_sb)):
                                nc.tensor.matmul(
                                    pss[(p, wi)][:],
                                    wsb[:, p, cc, :],
                                    xT_sb[:, cc, ts(b, 512)],
                                    start=(cc == 0), stop=False)
                    for p in range(2):
                        for wi, w9 in enumerate((wq9_sb, wk9_sb)):
                            nc.tensor.matmul(
                                pss[(p, wi)][:], w9[0:K9, p, :],
                                mrow[0:K9, ts(b, 512)],
                                start=False, stop=True)
                    psb = psB.tile([128, 512], F32, tag="psb")
                    nc.tensor.matmul(psb[:], ones1[0:1, :],
                                     rstd_row[0:1, ts(b, 512)],
                                     start=True, stop=True)
                    nc.scalar.copy(rstdb[b][:], psb[:])
                    for p in range(2):
                        for wi, dstT in enumerate((qT, kT)):
                            for j in range(2):
                                nc.vector.tensor_tensor(
                                    dstT[0:64, 2 * p + j, ts(b, 512)],
                                    pss[(p, wi)][64 * j:64 * j + 64, :],
                                    rstdb[b][64 * j:64 * j + 64, :],
                                    mybir.AluOpType.mult)

                for t in range(NTT):
                    psv = psV.tile([128, 256], F32, tag="v")
                    for cc in range(CCH):
                        nc.tensor.matmul(psv[:], xT_sb[:, cc, ts(t, 128)],
                                         wv_sb[:, cc, :],
                                         start=(cc == 0), stop=False)
                    nc.tensor.matmul(psv[:], mrow[0:K9, ts(t, 128)],
                                     wv9_sb[0:K9, :], start=False, stop=True)
                    for h in range(HL):
                        nc.vector.tensor_scalar(
                            Vh[:, t, h, 0:64], psv[:, ts(h, 64)],
                            rstds[t][:], ck32[:, t, h:h + 1],
                            mybir.AluOpType.mult, mybir.AluOpType.mult)

            # ---- attention + per-block proj/RS + deferred LN2 -----------
            with ExitStack() as ps_scope:
                psS = ps_scope.enter_context(
                    tc.tile_pool(name="psS", bufs=3, space="PSUM"))
                psO = ps_scope.enter_context(
                    tc.tile_pool(name="psO", bufs=3, space="PSUM"))

                def emit_proj(c):
                    # proj partial over local heads -> DRAM -> RS
                    for i2 in range(4):
                        for nb in range(2):
                            pp = psS.tile([128, 512], F32, tag="pp", bufs=2,
                                          name=f"pp_{c}_{nb}_{i2}")
                            for cc in range(2):
                                nc.tensor.matmul(pp[:],
                                                 OnT_cs[c][:, cc, ts(i2, 128)],
                                                 wp_sb[:, cc, ts(nb, 512)],
                                                 start=(cc == 0), stop=(cc == 1))
                            pjs = pipe.tile([128, 512], BF16, tag="pjs",
                                            bufs=2, name=f"pjs_{c}_{nb}_{i2}")
                            nc.vector.tensor_copy(pjs[:], pp[:])
                            nc.sync.dma_start(
                                rs_in[c][ts(i2, 128), ts(nb, 512)], pjs[:])
                    nc.gpsimd.collective_compute(
                        "ReduceScatter", mybir.AluOpType.add,
                        replica_groups=GROUPS,
                        ins=[rs_in[c].opt()], outs=[rs_out[c].opt()],
                    )

                def emit_ln2(c):
                    xrt = work.tile([128, C], F32, tag="xrt", bufs=2)
                    nc.sync.dma_start(xrt[:], xr_d[ts(c, 128), :])
                    rst = work.tile([128, C], BF16, tag="rst", bufs=2)
                    nc.sync.dma_start(rst[:], rs_out[c][:, :])
                    nc.vector.tensor_tensor(x2[:, c, :], rst[:], xrt[:],
                                            mybir.AluOpType.add)
                    bst = stats.tile([128, 2, 6], F32, tag="bst2")
                    for sg in range(2):
                        nc.vector.bn_stats(bst[:, sg, :], x2[:, c, ts(sg, 512)])
                    nc.vector.bn_aggr(mv2[c][:], bst[:])
                    std = stats.tile([128, 1], F32, tag="std2")
                    nc.scalar.activation(std[:], mv2[c][:, 1:2],
                                         mybir.ActivationFunctionType.Sqrt,
                                         bias=eps_ap[:])
                    rstd2 = stats.tile([128, 1], F32, tag="rstd2")
                    nc.vector.reciprocal(rstd2[:], std[:])
                    yb = work.tile([128, C], BF16, tag="yb", bufs=2)
                    nc.vector.tensor_scalar(yb[:], x2[:, c, :], mv2[c][:, 0:1],
                                            rstd2[:], mybir.AluOpType.subtract,
                                            mybir.AluOpType.mult)
                    for half in range(2):
                        pt = psS.tile([128, 4, 128], BF16, tag="pp", bufs=2,
                                      name=f"tr2_{c}_{half}")
                        for i in range(4):
                            cc = half * 4 + i
                            nc.tensor.transpose(pt[:, i, :], yb[:, ts(cc, 128)],
                                                ident[:])
                        nc.vector.tensor_copy(
                            y2T8[:, half * 4:(half + 1) * 4, ts(c, 128)], pt[:])

                npair = [0]
                pending = []
                OnT_cs = {}
                for c in (3, 2, 1, 0):
                    OnT_c = OnTp.tile([128, 2, 512], BF16, tag="OnT")
                    OnT_cs[c] = OnT_c
                    t_hi = 4 * c + 4
                    for p in range(2):
                        pos = [psO.tile([66, 512], F32, tag="po",
                                        name=f"po{j}_{c}_{p}") for j in range(2)]
                        for j in range(2):
                            h = 2 * p + j
                            po = pos[j]

                            def emit_s(t):
                                if t >= 4 * c:
                                    w = 512 - 128 * (t - 4 * c)
                                    diag = True
                                else:
                                    w, diag = 512, False
                                q0 = 512 * c + 512 - w
                                st = psS.tile([128, 512], F32, tag="s",
                                              name=f"s_{c}_{h}_{t}")
                                nc.tensor.matmul(st[:, 0:w],
                                                 kT[:, h, ts(t, 128)],
                                                 qT[:, h, q0:q0 + w],
                                                 start=True, stop=True)
                                pT = pipe.tile([128, 512], BF16, tag="pT",
                                               name=f"pT_{c}_{h}_{t}")
                                nc.scalar.activation(
                                    pT[:, 0:w], st[:, 0:w],
                                    mybir.ActivationFunctionType.Exp,
                                    bias=cb24[:])
                                if diag:
                                    nc.vector.tensor_tensor(
                                        pT[:, 0:w], pT[:, 0:w],
                                        tri[:, 0:w], mybir.AluOpType.mult)
                                return pT, w

                            pTs = {0: emit_s(0)}
                            if t_hi > 1:
                                pTs[1] = emit_s(1)
                            for t in range(t_hi):
                                pT, w = pTs.pop(t)
                                nc.tensor.matmul(po[:, 512 - w:512],
                                                 Vh[:, t, h, 0:66],
                                                 pT[:, 0:w], start=(t == 0),
                                                 stop=(t == t_hi - 1),
                                                 skip_group_check=True)
                                if t + 2 < t_hi:
                                    pTs[t + 2] = emit_s(t + 2)
                        # softmax denominators for the pair -> broadcast
                        dp = dpad2s[npair[0] % 2]
                        npair[0] += 1
                        nc.vector.tensor_copy(dp[0:1, :], pos[0][64:65, :])
                        nc.vector.tensor_copy(dp[64:65, :], pos[1][64:65, :])
                        rb = psS.tile([128, 512], F32, tag="s",
                                      name=f"rb_{c}_{p}")
                        nc.tensor.matmul(rb[:], sel2[:], dp[:],
                                         start=True, stop=True)
                        rec = recp.tile([128, 512], F32, tag="rec")
                        nc.vector.reciprocal(rec[:], rb[:])
                        nc.vector.tensor_tensor(OnT_c[0:64, p, :],
                                                pos[0][0:64, :], rec[0:64, :],
                                                mybir.AluOpType.mult)
                        nc.vector.tensor_tensor(OnT_c[64:128, p, :],
                                                pos[1][0:64, :],
                                                rec[64:128, :],
                                                mybir.AluOpType.mult)
                    emit_proj(c)
                    pending.append(c)
                    if len(pending) > 2:
                        emit_ln2(pending.pop(0))
                for c in pending:
                    emit_ln2(c)

        # ---- SwiGLU MLP (row-parallel, fp8 DoubleRow) -------------------
        with ExitStack() as mlp_scope:
            mpool = mlp_scope.enter_context(tc.tile_pool(name="mlp", bufs=1))
            wstream = mlp_scope.enter_context(tc.tile_pool(name="wstream", bufs=3))
            psC = mlp_scope.enter_context(
                tc.tile_pool(name="psC", bufs=2, space="PSUM"))

            gu = mpool.tile([128, MCH, ROWS], F8)
            db_sb = mpool.tile([128, 1024], F32)
            nc.gpsimd.dma_start(db_sb[:], db_d[:, :])
            wd_sb = mpool.tile([128, MCH, 1024], F8)
            for half in range(2):
                nc.gpsimd.dma_start(wd_sb[:, :, ts(half, 512)],
                                    wd_d[:, :, ts(half, 512)])

            for hc in range(MCH):
                wgt = wstream.tile([128, CCH, 128], F8, tag="wgt")
                nc.sync.dma_start(wgt[:], wg_d[:, hc, :, :])
                wut = wstream.tile([128, CCH, 128], F8, tag="wut")
                nc.sync.dma_start(wut[:], wu_d[:, hc, :, :])
                pg = psC.tile([128, 512], F32, tag="g")
                pu = psC.tile([128, 512], F32, tag="u")
                for c2 in range(CCH // 2):
                    nc.tensor.matmul(pg[:], wgt[:, 2 * c2:2 * c2 + 2, :],
                                     y2T8[:, 2 * c2:2 * c2 + 2, :],
                                     start=(c2 == 0), stop=(c2 == CCH // 2 - 1),
                                     perf_mode=DR)
                for c2 in range(CCH // 2):
                    nc.tensor.matmul(pu[:], wut[:, 2 * c2:2 * c2 + 2, :],
                                     y2T8[:, 2 * c2:2 * c2 + 2, :],
                                     start=(c2 == 0), stop=(c2 == CCH // 2 - 1),
                                     perf_mode=DR)
                gs = work.tile([128, 512], BF16, tag="gs")
                nc.scalar.activation(gs[:], pg[:],
                                     mybir.ActivationFunctionType.Silu,
                                     bias=bg_sb[:, hc:hc + 1])
                us = work.tile([128, 512], BF16, tag="us")
                nc.scalar.activation(us[:], pu[:],
                                     mybir.ActivationFunctionType.Identity,
                                     bias=bu_sb[:, hc:hc + 1])
                nc.vector.tensor_tensor(gu[:, hc, :], gs[:], us[:],
                                        mybir.AluOpType.mult)

            for tt in range(RT):
                pds = [psC.tile([128, 512], F32, tag="d", name=f"pd_{tt}_{nb}")
                       for nb in range(2)]
                for h2 in range(MCH // 2):
                    for nb in range(2):
                        nc.tensor.matmul(pds[nb][:],
                                         gu[:, 2 * h2:2 * h2 + 2, ts(tt, 128)],
                                         wd_sb[:, 2 * h2:2 * h2 + 2, ts(nb, 512)],
                                         start=(h2 == 0),
                                         stop=(h2 == MCH // 2 - 1),
                                         perf_mode=DR)
                for nb in range(2):
                    o1 = work.tile([128, 512], F32, tag="o1")
                    nc.vector.tensor_tensor(o1[:], pds[nb][:],
                                            x2[:, tt, ts(nb, 512)],
                                            mybir.AluOpType.add)
                    nc.vector.tensor_tensor(o1[:], o1[:],
                                            db_sb[:, ts(nb, 512)],
                                            mybir.AluOpType.add)
                    nc.sync.dma_start(out_d[ts(tt, 128), ts(nb, 512)], o1[:])

    _split_excess_waits(nc)
    return nc


# ----------------------------------------------------------------------------
# host-side input prep + launch
# ----------------------------------------------------------------------------
_cache = {}

def _get_nc(has_b1):
    if has_b1 not in _cache:
        _cache[has_b1] = _build(has_b1)
    return _cache[has_b1]


def _prep(x, ln1_g, ln1_b, qkv_w, qkv_b, proj_w, proj_b,
          ln2_g, ln2_b, gate_w, gate_b, up_w, up_b, down_w, down_b):
    x = np.asarray(x, np.float32)
    f = lambda a: np.asarray(a, np.float32)
    ln1_g, ln1_b, qkv_b, proj_b, ln2_g, ln2_b = map(f, (
        ln1_g, ln1_b, qkv_b, proj_b, ln2_g, ln2_b))
    qkv_w, proj_w, gate_w, gate_b, up_w, up_b, down_w, down_b = map(f, (
        qkv_w, proj_w, gate_w, gate_b, up_w, up_b, down_w, down_b))

    slopes = _slopes()

    # fold LN affines into the consuming matmuls
    w1 = qkv_w * ln1_g[:, None]
    b1 = ln1_b @ qkv_w + qkv_b              # [3C]
    wg_f = gate_w * ln2_g[:, None]
    bg_f = ln2_b @ gate_w + gate_b          # [HID]
    wu_f = up_w * ln2_g[:, None]
    bu_f = ln2_b @ up_w + up_b

    has_b1 = bool(np.any(b1 != 0.0))

    wgp = np.zeros((C, HIDP), np.float32); wgp[:, :HID] = wg_f
    wup = np.zeros((C, HIDP), np.float32); wup[:, :HID] = wu_f
    wdp = np.zeros((HIDP, 1024), np.float32); wdp[:HID] = down_w
    bgp = np.zeros(HIDP, np.float32); bgp[:HID] = bg_f
    bup = np.zeros(HIDP, np.float32); bup[:HID] = bu_f

    wg_dev = wgp.reshape(CCH, 128, MCH, 128).transpose(1, 2, 0, 3).astype(f8np)
    wu_dev = wup.reshape(CCH, 128, MCH, 128).transpose(1, 2, 0, 3).astype(f8np)
    wd_dev = wdp.reshape(MCH, 128, 1024).transpose(1, 0, 2).astype(f8np)
    bg_dev = bgp.reshape(MCH, 128).T.copy()
    bu_dev = bup.reshape(MCH, 128).T.copy()
    db_dev = np.broadcast_to(down_b, (128, 1024)).copy()

    # triangular diagonal mask (query >= key within a trimmed diag tile)
    pp_i = np.arange(128)[:, None]
    jj = np.arange(512)[None, :]
    tri_np = (jj >= pp_i).astype(bf16)       # [128, 512]

    sel2_np = np.zeros((128, 128), bf16)
    sel2_np[0, 0:64] = 1.0
    sel2_np[64, 64:128] = 1.0

    def qkv_w9(wcols, bcols):
        w9 = np.zeros((2, 2, 128), np.float32)
        w9[0] = wcols.sum(axis=0).reshape(2, 128)
        w9[1] = bcols.reshape(2, 128)
        return w9.astype(bf16)

    in_maps = []
    for core in range(N_CORES):
        b, g = core // TP, core % TP
        heads = range(4 * g, 4 * g + 4)
        qcols = np.concatenate([np.arange(h * D, (h + 1) * D) for h in heads])
        kcols = qcols + C
        vcols = qcols + 2 * C

        wq_cols = w1[:, qcols] * 0.125           # [C, 256]
        wk_cols = w1[:, kcols]
        wv_cols = w1[:, vcols]
        wq_dev = wq_cols.reshape(CCH, 128, 2, 128).transpose(1, 2, 0, 3).astype(bf16)
        wk_dev = wk_cols.reshape(CCH, 128, 2, 128).transpose(1, 2, 0, 3).astype(bf16)
        wv_dev = wv_cols.reshape(CCH, 128, 256).transpose(1, 0, 2).astype(bf16)
        wq9_dev = qkv_w9(wq_cols, b1[qcols] * 0.125)
        wk9_dev = qkv_w9(wk_cols, b1[kcols])
        wv9_dev = np.zeros((2, 256), np.float32)
        wv9_dev[0] = wv_cols.sum(axis=0)
        wv9_dev[1] = b1[vcols]
        wv9_dev = wv9_dev.astype(bf16)

        wp_rows = proj_w[qcols, :]                        # [256, 1024]
        wp_dev = wp_rows.reshape(2, 128, 1024).transpose(1, 0, 2).astype(bf16)

        # ALiBi key-side factors folded into V (and the denominator column)
        ck = np.zeros((128, NTT, HL), np.float64)
        for hl, h in enumerate(heads):
            sl = slopes[h]
            for t in range(NTT):
                ck[:, t, hl] = np.exp(-sl * (128 * t + np.arange(128)))
        ckf = ck.astype(np.float32)

        xb = x[b]                                # [T, C]
        xT_dev = np.ascontiguousarray(
            xb.T.reshape(CCH, 128, T).transpose(1, 0, 2)).astype(bf16)

        in_maps.append({
            "xT": xT_dev,
            "xtok": xb.astype(bf16),
            "x_rows": np.concatenate(
                [xb[512 * r + 128 * g:512 * r + 128 * g + 128]
                 for r in range(4)], axis=0) + proj_b[None, :],
            "wq": wq_dev, "wk": wk_dev, "wv": wv_dev,
            "wq9": wq9_dev, "wk9": wk9_dev, "wv9": wv9_dev,
            "wproj": wp_dev,
            "wg": wg_dev, "wu": wu_dev, "wd": wd_dev,
            "bg": bg_dev, "bu": bu_dev, "dbias": db_dev,
            "masks": tri_np, "ckb": ckf.astype(bf16), "ckf": ckf,
            "sel2": sel2_np,
        })

    return has_b1, in_maps


def _gather(results):
    out = np.empty((B, T, C), np.float32)
    for core in range(N_CORES):
        b, g = core // TP, core % TP
        orr = results[core]["out_rows"]
        for r in range(4):
            out[b, 512 * r + 128 * g:512 * r + 128 * g + 128] = \
                orr[128 * r:128 * (r + 1)]
    return out


def kernel(**inputs):
    has_b1, in_maps = _prep(**inputs)
    nc = _get_nc(has_b1)
    res = bass_utils.run_bass_kernel_spmd(
        nc, in_maps, core_ids=list(range(N_CORES)))
    return _gather(res.results)
